# revision 29
# baseline (speedup 1.0000x reference)
"""Trainium2 Bass kernel for the BN-attention module (nn_Attention).

Full inputs -> full output. Sharding: 8 cores = (batch b in 0..3) x
(head-group g in 0..1, 4 heads each). Each core computes its batch's
4-head attention and a partial output projection; the host sums the two
head-group partials per batch and adds the projection BN bias.

Numerics: BN scales are folded into the weights on the host. The Q/K
path (x, wq, wk, q, k) runs in fp16; exp input is fp32 PSUM and its
output bf16. All exp runs on the ScalarE: the exp instruction pace is
what the PE's ST stream is locked to (the two st PSUM buffers create a
write-after-read chain ST(mt+2) <- exp(mt)), so exp must stay on one
dedicated engine with deterministic FIFO latency -- offloading any of
it to the (queued) VectorE/GpSimd was measured to serialize the
pipeline.

Layout: attention is computed transposed, S^T = K^T Q with keys (m) on
partitions, so attn@V needs no transposes at all: V is produced
directly as vT[n,d] by the projection. Denominators are column sums:
exp tiles are pair-summed into t-tiles and pair-pair-summed into
u0..u3 on the VectorE (bf16 2x); the u-tiles and the trailing t8 are
column-summed by accumulating ones-matmuls (u0..u3 in one burst while
the last exp is in flight, filling the PE's wait at the pair
boundary). This keeps the denominator reduction off the ScalarE
entirely and replaces the deeper w-level VectorE folds of the previous
version with cheap PE work at the boundary. The finish (reciprocal on the
[1,JW] sums rows, GPSIMD partition_broadcast, normalize, fp16
bias+relu) is deferred into the next pair's loop, as is each chunk's
output projection, so the PE never stalls at pair/chunk boundaries.
Phase 1 is minimal (q/k chunk 0 only); the remaining q/k chunks and
all v projections are woven into chunk 0's m-tile loop, and input DMAs
are split across both HWDGE rings.
"""

import numpy as np

import concourse.bacc as bacc
import concourse.mybir as mybir
import concourse.tile as tile
from concourse.bass_utils import run_bass_kernel_spmd

# Problem dims (hardcoded per the spec)
B, C, H, W = 4, 256, 48, 48
N = H * W            # 2304
KD, NH, AR = 32, 8, 4
D = AR * KD          # 128 value dims per head
NHKD = NH * KD       # 256
DH = NH * D          # 1024
EPS = 1e-5

NHG = 4              # heads per core
J = 256              # (legacy) consts tile width
MT = 128             # m-tile (key tile)
NMT = N // MT        # 18

F32 = mybir.dt.float32
F32R = mybir.dt.float32r
BF16 = mybir.dt.bfloat16
FP16 = mybir.dt.float16
I16 = mybir.dt.int16
AF = mybir.ActivationFunctionType
OP = mybir.AluOpType

# Schraudolph bf16 fast-exp constants: bits = round(x*128/ln2 + 16256)
FE_A = float(128.0 / np.log(2.0))
FE_B = 16256.0

# chunks of 2304 by <=512 for the projection matmuls
CHUNKS_512 = [(off, min(512, N - off)) for off in range(0, N, 512)]

_CACHE = {}


def _build_program():
    nc = bacc.Bacc("TRN2", target_bir_lowering=False, debug=False)

    x_in = nc.dram_tensor("x_in", [C, N], FP16, kind="ExternalInput")
    wqt_d = nc.dram_tensor("wqt", [C, 128], FP16, kind="ExternalInput")
    wkt_d = nc.dram_tensor("wkt", [C, 128], FP16, kind="ExternalInput")
    wvt_d = nc.dram_tensor("wvt", [C, 512], FP16, kind="ExternalInput")
    wpt_d = nc.dram_tensor("wpt", [512, C], FP16, kind="ExternalInput")
    bq_d = nc.dram_tensor("bq", [128, 1], F32, kind="ExternalInput")
    bk_d = nc.dram_tensor("bk", [128, 1], F32, kind="ExternalInput")
    bv_d = nc.dram_tensor("bv", [512, 1], F32, kind="ExternalInput")
    consts_d = nc.dram_tensor("consts", [128, 128 + J], F32R, kind="ExternalInput")
    ones_bf_d = nc.dram_tensor("ones_bf", [128, 1], BF16, kind="ExternalInput")
    out_d = nc.dram_tensor("outp", [C, N], F32, kind="ExternalOutput")

    with tile.TileContext(nc) as tc:
        with nc.allow_low_precision(reason="16-bit matmul rounding is intentional"), \
             tc.tile_pool(name="const", bufs=1) as constp, \
             tc.tile_pool(name="qk", bufs=1) as qkp, \
             tc.tile_pool(name="vt", bufs=1) as vtp, \
             tc.tile_pool(name="pexp", bufs=1) as pexpp, \
             tc.tile_pool(name="rp", bufs=1) as rp, \
             tc.tile_pool(name="work", bufs=2) as workp:

            # ---------- constants / inputs ----------
            xf = [constp.tile([128, N], FP16, name=f"xf{c2}", tag=f"xf{c2}")
                  for c2 in range(2)]
            wqt, wkt, wvt = [], [], []
            nc.sync.dma_start(xf[0][:, 0:512], x_in.ap()[0:128, 0:512])
            for c2 in range(2):
                sl = slice(128 * c2, 128 * (c2 + 1))
                t = constp.tile([128, 128], FP16, name=f"wqt{c2}", tag=f"wqt{c2}")
                nc.scalar.dma_start(t[:], wqt_d.ap()[sl, :])
                wqt.append(t)
                t = constp.tile([128, 128], FP16, name=f"wkt{c2}", tag=f"wkt{c2}")
                nc.scalar.dma_start(t[:], wkt_d.ap()[sl, :])
                wkt.append(t)
            nc.sync.dma_start(xf[1][:, 0:512], x_in.ap()[128:256, 0:512])
            bq_t = constp.tile([128, 1], F32, name="bq_t", tag="bq_t")
            nc.scalar.dma_start(bq_t[:], bq_d.ap())
            bk_t = constp.tile([128, 1], F32, name="bk_t", tag="bk_t")
            nc.scalar.dma_start(bk_t[:], bk_d.ap())
            for c2 in range(2):
                t = constp.tile([128, 512], FP16, name=f"wvt{c2}",
                                tag=f"wvt{c2}")
                eng = nc.sync if c2 == 0 else nc.scalar
                eng.dma_start(t[:], wvt_d.ap()[128 * c2:128 * (c2 + 1), :])
                wvt.append(t)
            for off, w in CHUNKS_512:
                if off == 0:
                    continue
                nc.sync.dma_start(xf[0][:, off:off + w],
                                  x_in.ap()[0:128, off:off + w])
                nc.scalar.dma_start(xf[1][:, off:off + w],
                                    x_in.ap()[128:256, off:off + w])
            wpt = []
            for h in range(NHG):
                t = constp.tile([128, C], FP16, name=f"wpt{h}", tag=f"wpt{h}")
                eng = nc.sync if h % 2 == 0 else nc.scalar
                eng.dma_start(t[:], wpt_d.ap()[128 * h:128 * (h + 1), :])
                wpt.append(t)
            bv_t = []
            for h in range(NHG):
                t = constp.tile([128, 1], F32, name=f"bv{h}", tag=f"bv{h}")
                nc.sync.dma_start(t[:], bv_d.ap()[128 * h:128 * (h + 1), :])
                bv_t.append(t)
            ones_bf = constp.tile([128, 1], BF16, name="ones_bf", tag="ones_bf")
            nc.sync.dma_start(ones_bf[:], ones_bf_d.ap())

            q_all = qkp.tile([128, N], FP16, name="q_all", tag="q_all")
            k_all = qkp.tile([128, N], FP16, name="k_all", tag="k_all")
            vt_all = vtp.tile([128, NMT * 512], BF16, name="vt_all", tag="vt_all")

            # ---------- phase 1: q chunk 0 + k chunk 0 only ----------
            with tc.tile_pool(name="p1", bufs=4, space="PSUM") as p1:
                ps = p1.tile([128, 512], F32, name="qproj", tag="p1")
                for c2 in range(2):
                    nc.tensor.matmul(ps[:], wqt[c2][:], xf[c2][:, 0:512],
                                     start=(c2 == 0), stop=(c2 == 1))
                nc.vector.tensor_scalar_add(q_all[:, 0:512], ps[:], bq_t[:])
                ps = p1.tile([128, 512], F32, name="kproj", tag="p1")
                for c2 in range(2):
                    nc.tensor.matmul(ps[:], wkt[c2][:], xf[c2][:, 0:512],
                                     start=(c2 == 0), stop=(c2 == 1))
                nc.vector.tensor_scalar_add(k_all[:, 0:512], ps[:], bk_t[:])

            # ---------- phase 2: attention + output projection ----------
            NP = NMT // 2
            G2 = ((0, 1), (2, 3))
            JCHUNKS = [(0, 512, G2), (512, 512, G2), (1024, 512, G2),
                       (1536, 512, G2), (2048, 256, G2)]
            with tc.tile_pool(name="stp", bufs=1, space="PSUM") as stp, \
                 tc.tile_pool(name="xxp", bufs=1, space="PSUM") as xxp, \
                 tc.tile_pool(name="finp", bufs=1, space="PSUM") as finp:
                def emit_proj(joff_p, JW_p, r_p):
                    # output projection over all four heads of a finished
                    # chunk (deferred into the next chunk's loop so the PE
                    # never stalls on the finish chain at chunk boundaries)
                    for ct in range(2):
                        op_ps = finp.tile([128, 512], F32, name="op_ps",
                                          tag=f"f{ct}")
                        for h in range(NHG):
                            nc.tensor.matmul(
                                op_ps[:, 0:JW_p],
                                wpt[h][:, 128 * ct:128 * (ct + 1)],
                                r_p[h][:, 0:JW_p],
                                start=(h == 0), stop=(h == NHG - 1))
                        o_sb = workp.tile([128, 512], F32, name="o_sb",
                                          tag="o_sb")
                        nc.vector.tensor_copy(o_sb[:, 0:JW_p],
                                              op_ps[:, 0:JW_p])
                        nc.sync.dma_start(
                            out_d.ap()[128 * ct:128 * (ct + 1),
                                       joff_p:joff_p + JW_p],
                            o_sb[:, 0:JW_p])

                # q/k-projection schedules inside chunk 0 (mt -> chunk).
                QSCHED = {4: 1, 8: 2, 10: 3, 13: 4}
                KSCHED = {1: 1, 3: 2, 5: 3, 7: 4}

                pending = None     # (joff, JW, r_ts) of the previous chunk
                fin_prev = None    # previous pair's deferred finish closure
                pair_tasks = []
                for ci, (joff, JW, groups) in enumerate(JCHUNKS):
                    for gi, grp in enumerate(groups):
                        pair_tasks.append((ci, joff, JW, gi, grp,
                                           gi == len(groups) - 1))
                r_ts = None
                for ci, joff, JW, gi, grp, last_in_chunk in pair_tasks:
                    if True:
                        if gi == 0:
                            r_ts = [None] * NHG
                        G = len(grp)           # heads in this group
                        xxt = [xxp.tile([128, 512], F32, name=f"xx{j}",
                                        tag=f"xx{j}") for j in range(2)]
                        xx = {}
                        for i, h in enumerate(grp):
                            xx[h] = xxt[i][:, 0:JW]
                        pexp = [None] * NMT
                        tsum = [None] * NP
                        usum = [None] * 4
                        # all pairs except chunk0-gi0 (whose finp banks
                        # host the v/qk staging) column-sum each t-tile
                        # directly with ones-matmuls spread mid-loop: the
                        # extra ~0.4us of PE work every other step keeps
                        # the PE ahead of the exp pace so the HAM clock
                        # never re-throttles, and the u-level VectorE
                        # folds disappear.
                        spread_ones = not (ci == 0 and gi == 0)

                        def emit_qk_exp(mt):
                            # the G heads' matmuls run concurrently
                            # (different PE row groups); each head's slice
                            # stays within a psum bank.
                            moff = 128 * mt
                            pe = pexpp.tile([128, 1024], BF16, name="pe",
                                            tag="pe", bufs=8)
                            st = stp.tile([128, 1024], F32, name="st",
                                          tag=f"st{mt % 2}")
                            for i, h in enumerate(grp):
                                nc.tensor.matmul(
                                    st[:, 512 * i:512 * i + JW],
                                    k_all[32 * h:32 * (h + 1), moff:moff + 128],
                                    q_all[32 * h:32 * (h + 1), joff:joff + JW],
                                    start=True, stop=True,
                                    tile_position=(32 * h, 0))
                            if JW == 512:
                                nc.scalar.activation(pe[:, 0:1024],
                                                     st[:, 0:1024], AF.Exp)
                            else:
                                st_v = st.rearrange("p (a b) -> p a b",
                                                    b=512)[:, :, 0:JW]
                                pe_v = pe.rearrange("p (a b) -> p a b",
                                                    b=512)[:, :, 0:JW]
                                nc.scalar.activation(pe_v, st_v, AF.Exp)
                            pexp[mt] = pe

                        def emit_pv(mt, first=False):
                            # PV(1) executes first (psum reset); PV(0) joins
                            # late so the new pair's xx reset never waits on
                            # the previous pair's finish chain reading xx.
                            pe = pexp[mt]
                            for i, h in enumerate(grp):
                                nc.tensor.matmul(
                                    xx[h],
                                    vt_all[:, 512 * mt + 128 * h:
                                           512 * mt + 128 * (h + 1)],
                                    pe[:, 512 * i:512 * i + JW],
                                    start=first, stop=(mt == NMT - 1),
                                    skip_group_check=True)

                        def emit_tree(k):
                            # u-level folds as soon as both t inputs are
                            # ready; each u-tile's ones-matmul column-sums
                            # follow immediately, spread through the loop
                            # to fill PE bubbles (the accumulating sums
                            # tiles are allocated at u0). t8 joins in the
                            # deferred finish.
                            if k % 2 == 1 and k < 8:
                                u = k // 2
                                t = workp.tile([128, 1024], BF16,
                                               name=f"u{u}", tag=f"u{u}",
                                               bufs=1)
                                nc.vector.tensor_tensor(
                                    t[:], tsum[k - 1][:], tsum[k][:], OP.add)
                                usum[u] = t

                        # PV schedule: PVs trail their exp by two steps so
                        # they never wait on the exp semaphore; the first
                        # PVs start later still, with a 2-per-step ramp.
                        FS = 4 if JW == 512 else 6
                        pv_order = [1, 2, 0] + list(range(3, NMT))
                        pv_sched = {}
                        done = 0
                        for _mt in range(FS, NMT):
                            target = min(NMT, _mt - 1)
                            n = min(2 if _mt > FS else 1,
                                    max(0, target - done))
                            if n:
                                pv_sched[_mt] = pv_order[done:done + n]
                                done += n
                        pv_left = pv_order[done:]

                        sums_hs = [None, None]
                        for mt in range(NMT):
                            if mt == 0 and fin_prev is not None:
                                # the previous pair's finish matmuls are
                                # ready now; emitting them ahead of ST(0)
                                # fills the PE's wait on the previous pair's
                                # trailing exp (WAR on the st buffer)
                                fin_prev()
                                fin_prev = None
                            emit_qk_exp(mt)
                            if ci == 0 and gi == 0:
                                # v projection for m-tile mt, one step ahead
                                # of its PV consumer
                                ps_v = finp.tile([128, 512], F32, name="vps",
                                                 tag=f"f{mt % 2}")
                                for c2 in range(2):
                                    nc.tensor.matmul(
                                        ps_v[:],
                                        xf[c2][:, 128 * mt:128 * (mt + 1)],
                                        wvt[c2][:],
                                        start=(c2 == 0), stop=(c2 == 1))
                                nc.vector.tensor_copy(
                                    vt_all[:, 512 * mt:512 * (mt + 1)],
                                    ps_v[:])
                                qk_c = [(QSCHED, wqt, q_all, bq_t),
                                        (KSCHED, wkt, k_all, bk_t)]
                                for sched, wt, dst, bias in qk_c:
                                    if mt not in sched:
                                        continue
                                    qo = 512 * sched[mt]
                                    qw = min(512, N - qo)
                                    ps_q = finp.tile([128, 512], F32,
                                                     name="qps",
                                                     tag=f"f{(mt + 1) % 2}")
                                    for c2 in range(2):
                                        nc.tensor.matmul(
                                            ps_q[:, 0:qw], wt[c2][:],
                                            xf[c2][:, qo:qo + qw],
                                            start=(c2 == 0), stop=(c2 == 1))
                                    nc.vector.tensor_scalar_add(
                                        dst[:, qo:qo + qw],
                                        ps_q[:, 0:qw], bias[:])
                            for j in pv_sched.get(mt, ()):
                                emit_pv(j, first=(j == 1))
                            if mt == NMT - 1:
                                if spread_ones:
                                    # t6, t7 finish the pre-boundary sums;
                                    # t8 joins in the deferred finish
                                    for tk in (6, 7):
                                        for i in range(G):
                                            nc.tensor.matmul(
                                                sums_hs[i][:, 0:JW],
                                                ones_bf[:],
                                                tsum[tk][:, 512 * i:
                                                          512 * i + JW],
                                                start=False, stop=False)
                                else:
                                    # chunk0-gi0: u0..u3 in one burst
                                    for i in range(G):
                                        sums_hs[i] = finp.tile(
                                            [1, 512], F32, name="sums_h",
                                            tag=f"f{i % 2}")
                                        for uj in range(4):
                                            nc.tensor.matmul(
                                                sums_hs[i][:, 0:JW],
                                                ones_bf[:],
                                                usum[uj][:, 512 * i:
                                                          512 * i + JW],
                                                start=(uj == 0), stop=False)
                            if mt == 3 and gi == 0 and pending is not None:
                                emit_proj(*pending)
                                pending = None
                            if mt % 2 == 1 and mt < NMT - 1:
                                k = mt // 2
                                t = workp.tile([128, 1024], BF16,
                                               name=f"t{k}", tag=f"t{k}",
                                               bufs=1)
                                nc.vector.tensor_tensor(
                                    t[:], pexp[mt - 1][:], pexp[mt][:],
                                    OP.add)
                                tsum[k] = t
                                if not spread_ones:
                                    emit_tree(k)
                            if (spread_ones and mt >= 6 and mt % 2 == 0
                                    and mt <= 16):
                                # ones-matmul column-sums for t_{(mt-6)/2},
                                # 5 steps after its VectorE add (never
                                # exposed to DVE queue latency)
                                tk = (mt - 6) // 2
                                for i in range(G):
                                    if tk == 0:
                                        sums_hs[i] = finp.tile(
                                            [1, 512], F32, name="sums_h",
                                            tag=f"f{i % 2}")
                                    nc.tensor.matmul(
                                        sums_hs[i][:, 0:JW], ones_bf[:],
                                        tsum[tk][:, 512 * i:512 * i + JW],
                                        start=(tk == 0), stop=False)
                        for j in pv_left:
                            emit_pv(j, first=(j == 1))

                        def make_finish(grp=grp, xx=xx, pexp=pexp,
                                        sums_hs=sums_hs, r_out=r_ts, JW=JW):
                            def fin():
                                # finish: t8, denominators, normalize,
                                # bias+relu. Only the t8 ones-matmul waits
                                # on the trailing exp.
                                t8 = workp.tile([128, 1024], BF16,
                                                name="t8", tag="t8", bufs=1)
                                nc.vector.tensor_tensor(
                                    t8[:], pexp[16][:], pexp[17][:], OP.add)
                                for i, h in enumerate(grp):
                                    sums_h = sums_hs[i]
                                    nc.tensor.matmul(
                                        sums_h[:, 0:JW], ones_bf[:],
                                        t8[:, 512 * i:512 * i + JW],
                                        start=False, stop=True)
                                    s_inv = workp.tile([1, 512], F32,
                                                       name="s_inv",
                                                       tag="s_inv")
                                    nc.vector.reciprocal_approx_fast(
                                        s_inv[:, 0:JW], sums_h[:, 0:JW])
                                    inv_bc = workp.tile([128, 512], F32,
                                                        name="inv_bc",
                                                        tag="inv_bc")
                                    nc.gpsimd.partition_broadcast(
                                        inv_bc[:, 0:JW], s_inv[:, 0:JW])
                                    t_h = workp.tile([128, 512], F32,
                                                     name="t_h", tag="t_h")
                                    nc.vector.tensor_tensor(
                                        t_h[:, 0:JW], xx[h],
                                        inv_bc[:, 0:JW], OP.mult)
                                    r_h = rp.tile([128, 512], FP16,
                                                  name=f"r{h}", tag=f"r{h}")
                                    nc.vector.tensor_scalar(
                                        r_h[:, 0:JW], t_h[:, 0:JW],
                                        bv_t[h][:], 0.0, OP.add, OP.max)
                                    r_out[h] = r_h
                            return fin

                        fin_prev = make_finish()
                        if last_in_chunk:
                            pending = (joff, JW, r_ts)
                fin_prev()
                emit_proj(*pending)
    nc.compile()
    return nc


def _prep_inputs(x, wq, gq, bq, wk, gk, bk, wv, gv, bv, wp, gp, bp):
    """Fold BN scales into weights; build the 8 per-core input maps."""
    rs = np.float32(1.0 / np.sqrt(np.float32(1.0) + np.float32(EPS)))
    sq = (gq * rs).astype(np.float32)
    sk = (gk * rs).astype(np.float32)
    sv = (gv * rs).astype(np.float32)
    sp = (gp * rs).astype(np.float32)
    wq_f = (wq * sq[:, None]).astype(np.float16)
    wk_f = (wk * sk[:, None]).astype(np.float16)
    wv_f = (wv * sv[:, None]).astype(np.float16)
    wp_f = (wp * sp[:, None]).astype(np.float16)

    xf = np.ascontiguousarray(x.reshape(B, C, N).astype(np.float16))
    consts = np.zeros((128, 128 + J), dtype=np.float32)
    consts[:, 0:128] = 1.0
    import ml_dtypes
    ones_bf = np.ones((128, 1), dtype=ml_dtypes.bfloat16)
    in_maps = []
    for core in range(8):
        b, g = core // 2, core % 2
        qs = slice(128 * g, 128 * (g + 1))       # q/k rows for this head group
        vs = slice(512 * g, 512 * (g + 1))       # v rows / p cols for this group
        in_maps.append({
            "x_in": xf[b],
            "wqt": np.ascontiguousarray(wq_f[qs, :].T),
            "wkt": np.ascontiguousarray(wk_f[qs, :].T),
            "wvt": np.ascontiguousarray(wv_f[vs, :].T),
            "wpt": np.ascontiguousarray(wp_f[:, vs].T),
            "bq": np.ascontiguousarray(bq[qs].astype(np.float32)[:, None]),
            "bk": np.ascontiguousarray(bk[qs].astype(np.float32)[:, None]),
            "bv": np.ascontiguousarray(bv[vs].astype(np.float32)[:, None]),
            "consts": consts,
            "ones_bf": ones_bf,
        })
    return in_maps


def kernel(**inputs):
    if "nc" not in _CACHE:
        _CACHE["nc"] = _build_program()
    nc = _CACHE["nc"]

    in_maps = _prep_inputs(**{k: np.asarray(v) for k, v in inputs.items()})
    res = run_bass_kernel_spmd(nc, in_maps, list(range(8)))
    _CACHE["last_results"] = res

    bp = np.asarray(inputs["bp"]).astype(np.float32)
    out = np.empty((B, C, H, W), dtype=np.float32)
    for b in range(B):
        acc = res.results[2 * b]["outp"] + res.results[2 * b + 1]["outp"]
        acc = acc + bp[:, None]
        out[b] = acc.reshape(C, H, W)
    return out


# revision 30
# speedup vs baseline: 1.0346x; 1.0346x over previous
"""Trainium2 Bass kernel for the BN-attention module (nn_Attention).

Full inputs -> full output. Sharding: 8 cores = (batch b in 0..3) x
(head-group g in 0..1, 4 heads each). Each core computes its batch's
4-head attention and a partial output projection; the host sums the two
head-group partials per batch and adds the projection BN bias.

Numerics: BN scales are folded into the weights on the host. The Q/K
path (x, wq, wk, q, k) runs in fp16; exp input is fp32 PSUM and its
output bf16. All exp runs on the ScalarE: the exp instruction pace is
what the PE's ST stream is locked to (the two st PSUM buffers create a
write-after-read chain ST(mt+2) <- exp(mt)), so exp must stay on one
dedicated engine with deterministic FIFO latency -- offloading any of
it to the (queued) VectorE/GpSimd was measured to serialize the
pipeline.

Layout: attention is computed transposed, S^T = K^T Q with keys (m) on
partitions, so attn@V needs no transposes at all: V is produced
directly as vT[n,d] by the projection. Denominators are column sums:
exp tiles are pair-summed into t-tiles and pair-pair-summed into
u0..u3 on the VectorE (bf16 2x); the u-tiles and the trailing t8 are
column-summed by accumulating ones-matmuls (u0..u3 in one burst while
the last exp is in flight, filling the PE's wait at the pair
boundary). This keeps the denominator reduction off the ScalarE
entirely and replaces the deeper w-level VectorE folds of the previous
version with cheap PE work at the boundary. The finish (reciprocal on the
[1,JW] sums rows, GPSIMD partition_broadcast, normalize, fp16
bias+relu) is deferred into the next pair's loop, as is each chunk's
output projection, so the PE never stalls at pair/chunk boundaries.
Phase 1 is minimal (q/k chunk 0 only); the remaining q/k chunks and
all v projections are woven into chunk 0's m-tile loop, and input DMAs
are split across both HWDGE rings.
"""

import numpy as np

import concourse.bacc as bacc
import concourse.mybir as mybir
import concourse.tile as tile
from concourse.bass_utils import run_bass_kernel_spmd

# Problem dims (hardcoded per the spec)
B, C, H, W = 4, 256, 48, 48
N = H * W            # 2304
KD, NH, AR = 32, 8, 4
D = AR * KD          # 128 value dims per head
NHKD = NH * KD       # 256
DH = NH * D          # 1024
EPS = 1e-5

NHG = 4              # heads per core
J = 256              # (legacy) consts tile width
MT = 128             # m-tile (key tile)
NMT = N // MT        # 18

F32 = mybir.dt.float32
F32R = mybir.dt.float32r
BF16 = mybir.dt.bfloat16
FP16 = mybir.dt.float16
I16 = mybir.dt.int16
AF = mybir.ActivationFunctionType
OP = mybir.AluOpType

# Schraudolph bf16 fast-exp constants: bits = round(x*128/ln2 + 16256)
FE_A = float(128.0 / np.log(2.0))
FE_B = 16256.0

# chunks of 2304 by <=512 for the projection matmuls
CHUNKS_512 = [(off, min(512, N - off)) for off in range(0, N, 512)]

_CACHE = {}


def _build_program():
    nc = bacc.Bacc("TRN2", target_bir_lowering=False, debug=False)

    x_in = nc.dram_tensor("x_in", [C, N], FP16, kind="ExternalInput")
    wqt_d = nc.dram_tensor("wqt", [C, 128], FP16, kind="ExternalInput")
    wkt_d = nc.dram_tensor("wkt", [C, 128], FP16, kind="ExternalInput")
    wvt_d = nc.dram_tensor("wvt", [C, 512], FP16, kind="ExternalInput")
    wpt_d = nc.dram_tensor("wpt", [512, C], FP16, kind="ExternalInput")
    bq_d = nc.dram_tensor("bq", [128, 1], F32, kind="ExternalInput")
    bk_d = nc.dram_tensor("bk", [128, 1], F32, kind="ExternalInput")
    bv_d = nc.dram_tensor("bv", [512, 1], F32, kind="ExternalInput")
    consts_d = nc.dram_tensor("consts", [128, 128 + J], F32R, kind="ExternalInput")
    ones_bf_d = nc.dram_tensor("ones_bf", [128, 1], BF16, kind="ExternalInput")
    out_d = nc.dram_tensor("outp", [C, N], F32, kind="ExternalOutput")

    with tile.TileContext(nc) as tc:
        with nc.allow_low_precision(reason="16-bit matmul rounding is intentional"), \
             tc.tile_pool(name="const", bufs=1) as constp, \
             tc.tile_pool(name="qk", bufs=1) as qkp, \
             tc.tile_pool(name="vt", bufs=1) as vtp, \
             tc.tile_pool(name="pexp", bufs=1) as pexpp, \
             tc.tile_pool(name="rp", bufs=1) as rp, \
             tc.tile_pool(name="work", bufs=2) as workp:

            # ---------- constants / inputs ----------
            xf = [constp.tile([128, N], FP16, name=f"xf{c2}", tag=f"xf{c2}")
                  for c2 in range(2)]
            wqt, wkt, wvt = [], [], []
            nc.sync.dma_start(xf[0][:, 0:512], x_in.ap()[0:128, 0:512])
            for c2 in range(2):
                sl = slice(128 * c2, 128 * (c2 + 1))
                t = constp.tile([128, 128], FP16, name=f"wqt{c2}", tag=f"wqt{c2}")
                nc.scalar.dma_start(t[:], wqt_d.ap()[sl, :])
                wqt.append(t)
                t = constp.tile([128, 128], FP16, name=f"wkt{c2}", tag=f"wkt{c2}")
                nc.scalar.dma_start(t[:], wkt_d.ap()[sl, :])
                wkt.append(t)
            nc.sync.dma_start(xf[1][:, 0:512], x_in.ap()[128:256, 0:512])
            bq_t = constp.tile([128, 1], F32, name="bq_t", tag="bq_t")
            nc.scalar.dma_start(bq_t[:], bq_d.ap())
            bk_t = constp.tile([128, 1], F32, name="bk_t", tag="bk_t")
            nc.scalar.dma_start(bk_t[:], bk_d.ap())
            for c2 in range(2):
                t = constp.tile([128, 512], FP16, name=f"wvt{c2}",
                                tag=f"wvt{c2}")
                eng = nc.sync if c2 == 0 else nc.scalar
                eng.dma_start(t[:], wvt_d.ap()[128 * c2:128 * (c2 + 1), :])
                wvt.append(t)
            for off, w in CHUNKS_512:
                if off == 0:
                    continue
                nc.sync.dma_start(xf[0][:, off:off + w],
                                  x_in.ap()[0:128, off:off + w])
                nc.scalar.dma_start(xf[1][:, off:off + w],
                                    x_in.ap()[128:256, off:off + w])
            wpt = []
            for h in range(NHG):
                t = constp.tile([128, C], FP16, name=f"wpt{h}", tag=f"wpt{h}")
                eng = nc.sync if h % 2 == 0 else nc.scalar
                eng.dma_start(t[:], wpt_d.ap()[128 * h:128 * (h + 1), :])
                wpt.append(t)
            bv_t = []
            for h in range(NHG):
                t = constp.tile([128, 1], F32, name=f"bv{h}", tag=f"bv{h}")
                nc.sync.dma_start(t[:], bv_d.ap()[128 * h:128 * (h + 1), :])
                bv_t.append(t)
            ones_bf = constp.tile([128, 1], BF16, name="ones_bf", tag="ones_bf")
            nc.sync.dma_start(ones_bf[:], ones_bf_d.ap())

            q_all = qkp.tile([128, N], FP16, name="q_all", tag="q_all")
            k_all = qkp.tile([128, N], FP16, name="k_all", tag="k_all")
            vt_all = vtp.tile([128, NMT * 512], BF16, name="vt_all", tag="vt_all")

            # ---------- phase 1: q chunk 0 + k chunk 0 only ----------
            with tc.tile_pool(name="p1", bufs=4, space="PSUM") as p1:
                ps = p1.tile([128, 512], F32, name="qproj", tag="p1")
                for c2 in range(2):
                    nc.tensor.matmul(ps[:], wqt[c2][:], xf[c2][:, 0:512],
                                     start=(c2 == 0), stop=(c2 == 1))
                nc.vector.tensor_scalar_add(q_all[:, 0:512], ps[:], bq_t[:])
                ps = p1.tile([128, 512], F32, name="kproj", tag="p1")
                for c2 in range(2):
                    nc.tensor.matmul(ps[:], wkt[c2][:], xf[c2][:, 0:512],
                                     start=(c2 == 0), stop=(c2 == 1))
                nc.vector.tensor_scalar_add(k_all[:, 0:512], ps[:], bk_t[:])

            # ---------- phase 2: attention + output projection ----------
            NP = NMT // 2
            G2 = ((0, 1), (2, 3))
            JCHUNKS = [(0, 512, G2), (512, 512, G2), (1024, 512, G2),
                       (1536, 512, G2), (2048, 256, G2)]
            with tc.tile_pool(name="stp", bufs=1, space="PSUM") as stp, \
                 tc.tile_pool(name="xxp", bufs=1, space="PSUM") as xxp, \
                 tc.tile_pool(name="finp", bufs=1, space="PSUM") as finp:
                def emit_proj(joff_p, JW_p, r_p):
                    # output projection over all four heads of a finished
                    # chunk (deferred into the next chunk's loop so the PE
                    # never stalls on the finish chain at chunk boundaries)
                    for ct in range(2):
                        op_ps = finp.tile([128, 512], F32, name="op_ps",
                                          tag=f"f{ct}")
                        for h in range(NHG):
                            nc.tensor.matmul(
                                op_ps[:, 0:JW_p],
                                wpt[h][:, 128 * ct:128 * (ct + 1)],
                                r_p[h][:, 0:JW_p],
                                start=(h == 0), stop=(h == NHG - 1))
                        o_sb = workp.tile([128, 512], F32, name="o_sb",
                                          tag="o_sb")
                        nc.vector.tensor_copy(o_sb[:, 0:JW_p],
                                              op_ps[:, 0:JW_p])
                        nc.sync.dma_start(
                            out_d.ap()[128 * ct:128 * (ct + 1),
                                       joff_p:joff_p + JW_p],
                            o_sb[:, 0:JW_p])

                # q/k-projection schedules inside chunk 0 (mt -> chunk).
                QSCHED = {4: 1, 8: 2, 10: 3, 13: 4}
                KSCHED = {1: 1, 3: 2, 5: 3, 7: 4}

                pending = None     # (joff, JW, r_ts) of the previous chunk
                fin_prev = None    # previous pair's deferred finish closure
                pair_tasks = []
                for ci, (joff, JW, groups) in enumerate(JCHUNKS):
                    for gi, grp in enumerate(groups):
                        pair_tasks.append((ci, joff, JW, gi, grp,
                                           gi == len(groups) - 1))
                r_ts = None
                for ci, joff, JW, gi, grp, last_in_chunk in pair_tasks:
                    if True:
                        if gi == 0:
                            r_ts = [None] * NHG
                        G = len(grp)           # heads in this group
                        xxt = [xxp.tile([128, 512], F32, name=f"xx{j}",
                                        tag=f"xx{j}") for j in range(2)]
                        xx = {}
                        for i, h in enumerate(grp):
                            xx[h] = xxt[i][:, 0:JW]
                        pexp = [None] * NMT
                        tsum = [None] * NP
                        usum = [None] * 4

                        def emit_qk_exp(mt):
                            # the G heads' matmuls run concurrently
                            # (different PE row groups); each head's slice
                            # stays within a psum bank.
                            moff = 128 * mt
                            pe = pexpp.tile([128, 1024], BF16, name="pe",
                                            tag="pe", bufs=8)
                            st = stp.tile([128, 1024], F32, name="st",
                                          tag=f"st{mt % 2}")
                            for i, h in enumerate(grp):
                                nc.tensor.matmul(
                                    st[:, 512 * i:512 * i + JW],
                                    k_all[32 * h:32 * (h + 1), moff:moff + 128],
                                    q_all[32 * h:32 * (h + 1), joff:joff + JW],
                                    start=True, stop=True,
                                    tile_position=(32 * h, 0))
                            if JW == 512:
                                nc.scalar.activation(pe[:, 0:1024],
                                                     st[:, 0:1024], AF.Exp)
                            else:
                                st_v = st.rearrange("p (a b) -> p a b",
                                                    b=512)[:, :, 0:JW]
                                pe_v = pe.rearrange("p (a b) -> p a b",
                                                    b=512)[:, :, 0:JW]
                                nc.scalar.activation(pe_v, st_v, AF.Exp)
                            pexp[mt] = pe

                        def emit_pv(mt, first=False):
                            # PV(1) executes first (psum reset); PV(0) joins
                            # late so the new pair's xx reset never waits on
                            # the previous pair's finish chain reading xx.
                            pe = pexp[mt]
                            for i, h in enumerate(grp):
                                nc.tensor.matmul(
                                    xx[h],
                                    vt_all[:, 512 * mt + 128 * h:
                                           512 * mt + 128 * (h + 1)],
                                    pe[:, 512 * i:512 * i + JW],
                                    start=first, stop=(mt == NMT - 1),
                                    skip_group_check=True)

                        def emit_tree(k):
                            # u-level folds as soon as both t inputs are
                            # ready; each u-tile's ones-matmul column-sums
                            # follow immediately, spread through the loop
                            # to fill PE bubbles (the accumulating sums
                            # tiles are allocated at u0). t8 joins in the
                            # deferred finish.
                            if k % 2 == 1 and k < 8:
                                u = k // 2
                                t = workp.tile([128, 1024], BF16,
                                               name=f"u{u}", tag=f"u{u}",
                                               bufs=1)
                                nc.vector.tensor_tensor(
                                    t[:], tsum[k - 1][:], tsum[k][:], OP.add)
                                usum[u] = t

                        # PV schedule: PVs trail their exp by two steps so
                        # they never wait on the exp semaphore; the first
                        # PVs start later still, with a 2-per-step ramp.
                        FS = 4 if JW == 512 else 6
                        pv_order = [1, 2, 0] + list(range(3, NMT))
                        pv_sched = {}
                        done = 0
                        for _mt in range(FS, NMT):
                            target = min(NMT, _mt - 1)
                            n = min(2 if _mt > FS else 1,
                                    max(0, target - done))
                            if n:
                                pv_sched[_mt] = pv_order[done:done + n]
                                done += n
                        pv_left = pv_order[done:]

                        sums_hs = [None, None]
                        for mt in range(NMT):
                            if mt == 0 and fin_prev is not None:
                                # the previous pair's finish matmuls are
                                # ready now; emitting them ahead of ST(0)
                                # fills the PE's wait on the previous pair's
                                # trailing exp (WAR on the st buffer)
                                fin_prev()
                                fin_prev = None
                            emit_qk_exp(mt)
                            if ci == 0 and gi == 0:
                                # v projection for m-tile mt, one step ahead
                                # of its PV consumer
                                ps_v = finp.tile([128, 512], F32, name="vps",
                                                 tag=f"f{mt % 2}")
                                for c2 in range(2):
                                    nc.tensor.matmul(
                                        ps_v[:],
                                        xf[c2][:, 128 * mt:128 * (mt + 1)],
                                        wvt[c2][:],
                                        start=(c2 == 0), stop=(c2 == 1))
                                nc.vector.tensor_copy(
                                    vt_all[:, 512 * mt:512 * (mt + 1)],
                                    ps_v[:])
                                qk_c = [(QSCHED, wqt, q_all, bq_t),
                                        (KSCHED, wkt, k_all, bk_t)]
                                for sched, wt, dst, bias in qk_c:
                                    if mt not in sched:
                                        continue
                                    qo = 512 * sched[mt]
                                    qw = min(512, N - qo)
                                    ps_q = finp.tile([128, 512], F32,
                                                     name="qps",
                                                     tag=f"f{(mt + 1) % 2}")
                                    for c2 in range(2):
                                        nc.tensor.matmul(
                                            ps_q[:, 0:qw], wt[c2][:],
                                            xf[c2][:, qo:qo + qw],
                                            start=(c2 == 0), stop=(c2 == 1))
                                    nc.vector.tensor_scalar_add(
                                        dst[:, qo:qo + qw],
                                        ps_q[:, 0:qw], bias[:])
                            for j in pv_sched.get(mt, ()):
                                emit_pv(j, first=(j == 1))
                            if mt == NMT - 1:
                                # u0..u3 cover m-tiles 0..15; start the
                                # denominator accumulation while the last
                                # exp is still in flight.
                                for i in range(G):
                                    sums_hs[i] = finp.tile(
                                        [1, 512], F32, name="sums_h",
                                        tag=f"f{i % 2}")
                                    for uj in range(4):
                                        nc.tensor.matmul(
                                            sums_hs[i][:, 0:JW], ones_bf[:],
                                            usum[uj][:, 512 * i:512 * i + JW],
                                            start=(uj == 0), stop=False)
                            if mt == 3 and gi == 0 and pending is not None:
                                emit_proj(*pending)
                                pending = None
                            if mt % 2 == 1 and mt < NMT - 1:
                                k = mt // 2
                                t = workp.tile([128, 1024], BF16,
                                               name=f"t{k}", tag=f"t{k}",
                                               bufs=1)
                                nc.vector.tensor_tensor(
                                    t[:], pexp[mt - 1][:], pexp[mt][:],
                                    OP.add)
                                tsum[k] = t
                                emit_tree(k)
                        for j in pv_left:
                            emit_pv(j, first=(j == 1))

                        def make_finish(grp=grp, xx=xx, pexp=pexp,
                                        sums_hs=sums_hs, r_out=r_ts, JW=JW):
                            def fin():
                                # finish: t8, denominators, normalize,
                                # bias+relu. Only the t8 ones-matmul waits
                                # on the trailing exp.
                                t8 = workp.tile([128, 1024], BF16,
                                                name="t8", tag="t8", bufs=1)
                                nc.vector.tensor_tensor(
                                    t8[:], pexp[16][:], pexp[17][:], OP.add)
                                for i, h in enumerate(grp):
                                    sums_h = sums_hs[i]
                                    nc.tensor.matmul(
                                        sums_h[:, 0:JW], ones_bf[:],
                                        t8[:, 512 * i:512 * i + JW],
                                        start=False, stop=True)
                                    s_inv = workp.tile([1, 512], F32,
                                                       name="s_inv",
                                                       tag="s_inv")
                                    nc.vector.reciprocal_approx_fast(
                                        s_inv[:, 0:JW], sums_h[:, 0:JW])
                                    inv_bc = workp.tile([128, 512], F32,
                                                        name="inv_bc",
                                                        tag="inv_bc")
                                    nc.gpsimd.partition_broadcast(
                                        inv_bc[:, 0:JW], s_inv[:, 0:JW])
                                    t_h = workp.tile([128, 512], F32,
                                                     name="t_h", tag="t_h")
                                    nc.vector.tensor_tensor(
                                        t_h[:, 0:JW], xx[h],
                                        inv_bc[:, 0:JW], OP.mult)
                                    r_h = rp.tile([128, 512], FP16,
                                                  name=f"r{h}", tag=f"r{h}")
                                    nc.vector.tensor_scalar(
                                        r_h[:, 0:JW], t_h[:, 0:JW],
                                        bv_t[h][:], 0.0, OP.add, OP.max)
                                    r_out[h] = r_h
                            return fin

                        fin_prev = make_finish()
                        if last_in_chunk:
                            pending = (joff, JW, r_ts)
                fin_prev()
                emit_proj(*pending)
    nc.compile()
    return nc


def _prep_inputs(x, wq, gq, bq, wk, gk, bk, wv, gv, bv, wp, gp, bp):
    """Fold BN scales into weights; build the 8 per-core input maps."""
    rs = np.float32(1.0 / np.sqrt(np.float32(1.0) + np.float32(EPS)))
    sq = (gq * rs).astype(np.float32)
    sk = (gk * rs).astype(np.float32)
    sv = (gv * rs).astype(np.float32)
    sp = (gp * rs).astype(np.float32)
    wq_f = (wq * sq[:, None]).astype(np.float16)
    wk_f = (wk * sk[:, None]).astype(np.float16)
    wv_f = (wv * sv[:, None]).astype(np.float16)
    wp_f = (wp * sp[:, None]).astype(np.float16)

    xf = np.ascontiguousarray(x.reshape(B, C, N).astype(np.float16))
    consts = np.zeros((128, 128 + J), dtype=np.float32)
    consts[:, 0:128] = 1.0
    import ml_dtypes
    ones_bf = np.ones((128, 1), dtype=ml_dtypes.bfloat16)
    in_maps = []
    for core in range(8):
        b, g = core // 2, core % 2
        qs = slice(128 * g, 128 * (g + 1))       # q/k rows for this head group
        vs = slice(512 * g, 512 * (g + 1))       # v rows / p cols for this group
        in_maps.append({
            "x_in": xf[b],
            "wqt": np.ascontiguousarray(wq_f[qs, :].T),
            "wkt": np.ascontiguousarray(wk_f[qs, :].T),
            "wvt": np.ascontiguousarray(wv_f[vs, :].T),
            "wpt": np.ascontiguousarray(wp_f[:, vs].T),
            "bq": np.ascontiguousarray(bq[qs].astype(np.float32)[:, None]),
            "bk": np.ascontiguousarray(bk[qs].astype(np.float32)[:, None]),
            "bv": np.ascontiguousarray(bv[vs].astype(np.float32)[:, None]),
            "consts": consts,
            "ones_bf": ones_bf,
        })
    return in_maps


def kernel(**inputs):
    if "nc" not in _CACHE:
        _CACHE["nc"] = _build_program()
    nc = _CACHE["nc"]

    in_maps = _prep_inputs(**{k: np.asarray(v) for k, v in inputs.items()})
    res = run_bass_kernel_spmd(nc, in_maps, list(range(8)))
    _CACHE["last_results"] = res

    bp = np.asarray(inputs["bp"]).astype(np.float32)
    out = np.empty((B, C, H, W), dtype=np.float32)
    for b in range(B):
        acc = res.results[2 * b]["outp"] + res.results[2 * b + 1]["outp"]
        acc = acc + bp[:, None]
        out[b] = acc.reshape(C, H, W)
    return out


# revision 34
# speedup vs baseline: 1.0604x; 1.0250x over previous
"""Trainium2 Bass kernel for the BN-attention module (nn_Attention).

Full inputs -> full output. Sharding: 8 cores = (batch b in 0..3) x
(head-group g in 0..1, 4 heads each). Each core computes its batch's
4-head attention and a partial output projection; the host sums the two
head-group partials per batch and adds the projection BN bias.

Numerics: BN scales are folded into the weights on the host. The Q/K
path (x, wq, wk, q, k) runs in fp16; exp input is fp32 PSUM and its
output bf16. All exp runs on the ScalarE: the exp instruction pace is
what the PE's ST stream is locked to (the two st PSUM buffers create a
write-after-read chain ST(mt+2) <- exp(mt)), so exp must stay on one
dedicated engine with deterministic FIFO latency -- offloading any of
it to the (queued) VectorE/GpSimd was measured to serialize the
pipeline.

Layout: attention is computed transposed, S^T = K^T Q with keys (m) on
partitions, so attn@V needs no transposes at all: V is produced
directly as vT[n,d] by the projection. Denominators are column sums:
exp tiles are pair-summed into t-tiles and pair-pair-summed into
u0..u3 on the VectorE (bf16 2x); the u-tiles and the trailing t8 are
column-summed by accumulating ones-matmuls (u0..u3 in one burst while
the last exp is in flight, filling the PE's wait at the pair
boundary). This keeps the denominator reduction off the ScalarE
entirely and replaces the deeper w-level VectorE folds of the previous
version with cheap PE work at the boundary. The finish (reciprocal on the
[1,JW] sums rows, GPSIMD partition_broadcast, normalize, fp16
bias+relu) is deferred into the next pair's loop, as is each chunk's
output projection, so the PE never stalls at pair/chunk boundaries.
Phase 1 is minimal (q/k chunk 0 only); the remaining q/k chunks and
all v projections are woven into chunk 0's m-tile loop, and input DMAs
are split across both HWDGE rings.
"""

import numpy as np

import concourse.bacc as bacc
import concourse.mybir as mybir
import concourse.tile as tile
from concourse.bass_utils import run_bass_kernel_spmd

# Problem dims (hardcoded per the spec)
B, C, H, W = 4, 256, 48, 48
N = H * W            # 2304
KD, NH, AR = 32, 8, 4
D = AR * KD          # 128 value dims per head
NHKD = NH * KD       # 256
DH = NH * D          # 1024
EPS = 1e-5

NHG = 4              # heads per core
J = 256              # (legacy) consts tile width
MT = 128             # m-tile (key tile)
NMT = N // MT        # 18

F32 = mybir.dt.float32
F32R = mybir.dt.float32r
BF16 = mybir.dt.bfloat16
FP16 = mybir.dt.float16
I16 = mybir.dt.int16
AF = mybir.ActivationFunctionType
OP = mybir.AluOpType

# Schraudolph bf16 fast-exp constants: bits = round(x*128/ln2 + 16256)
FE_A = float(128.0 / np.log(2.0))
FE_B = 16256.0

# chunks of 2304 by <=512 for the projection matmuls
CHUNKS_512 = [(off, min(512, N - off)) for off in range(0, N, 512)]

_CACHE = {}


def _build_program():
    nc = bacc.Bacc("TRN2", target_bir_lowering=False, debug=False)

    x_in = nc.dram_tensor("x_in", [C, N], FP16, kind="ExternalInput")
    wqt_d = nc.dram_tensor("wqt", [C, 128], FP16, kind="ExternalInput")
    wkt_d = nc.dram_tensor("wkt", [C, 128], FP16, kind="ExternalInput")
    wvt_d = nc.dram_tensor("wvt", [C, 512], FP16, kind="ExternalInput")
    wpt_d = nc.dram_tensor("wpt", [512, C], FP16, kind="ExternalInput")
    bq_d = nc.dram_tensor("bq", [128, 1], F32, kind="ExternalInput")
    bk_d = nc.dram_tensor("bk", [128, 1], F32, kind="ExternalInput")
    bv_d = nc.dram_tensor("bv", [512, 1], F32, kind="ExternalInput")
    consts_d = nc.dram_tensor("consts", [128, 128 + J], F32R, kind="ExternalInput")
    ones_bf_d = nc.dram_tensor("ones_bf", [128, 1], BF16, kind="ExternalInput")
    out_d = nc.dram_tensor("outp", [C, N], F32, kind="ExternalOutput")

    with tile.TileContext(nc) as tc:
        with nc.allow_low_precision(reason="16-bit matmul rounding is intentional"), \
             tc.tile_pool(name="const", bufs=1) as constp, \
             tc.tile_pool(name="qk", bufs=1) as qkp, \
             tc.tile_pool(name="vt", bufs=1) as vtp, \
             tc.tile_pool(name="pexp", bufs=1) as pexpp, \
             tc.tile_pool(name="rp", bufs=1) as rp, \
             tc.tile_pool(name="work", bufs=2) as workp:

            # ---------- constants / inputs ----------
            xf = [constp.tile([128, N], FP16, name=f"xf{c2}", tag=f"xf{c2}")
                  for c2 in range(2)]
            wqt, wkt, wvt = [], [], []
            nc.sync.dma_start(xf[0][:, 0:512], x_in.ap()[0:128, 0:512])
            for c2 in range(2):
                sl = slice(128 * c2, 128 * (c2 + 1))
                t = constp.tile([128, 128], FP16, name=f"wqt{c2}", tag=f"wqt{c2}")
                nc.scalar.dma_start(t[:], wqt_d.ap()[sl, :])
                wqt.append(t)
                t = constp.tile([128, 128], FP16, name=f"wkt{c2}", tag=f"wkt{c2}")
                nc.scalar.dma_start(t[:], wkt_d.ap()[sl, :])
                wkt.append(t)
            nc.sync.dma_start(xf[1][:, 0:512], x_in.ap()[128:256, 0:512])
            bq_t = constp.tile([128, 1], F32, name="bq_t", tag="bq_t")
            nc.scalar.dma_start(bq_t[:], bq_d.ap())
            bk_t = constp.tile([128, 1], F32, name="bk_t", tag="bk_t")
            nc.scalar.dma_start(bk_t[:], bk_d.ap())
            for c2 in range(2):
                t = constp.tile([128, 512], FP16, name=f"wvt{c2}",
                                tag=f"wvt{c2}")
                eng = nc.sync if c2 == 0 else nc.scalar
                eng.dma_start(t[:], wvt_d.ap()[128 * c2:128 * (c2 + 1), :])
                wvt.append(t)
            for off, w in CHUNKS_512:
                if off == 0:
                    continue
                nc.sync.dma_start(xf[0][:, off:off + w],
                                  x_in.ap()[0:128, off:off + w])
                nc.scalar.dma_start(xf[1][:, off:off + w],
                                    x_in.ap()[128:256, off:off + w])
            wpt = []
            for h in range(NHG):
                t = constp.tile([128, C], FP16, name=f"wpt{h}", tag=f"wpt{h}")
                eng = nc.sync if h % 2 == 0 else nc.scalar
                eng.dma_start(t[:], wpt_d.ap()[128 * h:128 * (h + 1), :])
                wpt.append(t)
            bv_t = []
            for h in range(NHG):
                t = constp.tile([128, 1], F32, name=f"bv{h}", tag=f"bv{h}")
                nc.sync.dma_start(t[:], bv_d.ap()[128 * h:128 * (h + 1), :])
                bv_t.append(t)
            ones_bf = constp.tile([128, 1], BF16, name="ones_bf", tag="ones_bf")
            nc.sync.dma_start(ones_bf[:], ones_bf_d.ap())

            q_all = qkp.tile([128, N], FP16, name="q_all", tag="q_all")
            k_all = qkp.tile([128, N], FP16, name="k_all", tag="k_all")
            # heads 2,3 replicated on partitions 0:64: the merged tail
            # chunk runs them in PE row groups 0,1 so they serialize
            # behind heads 0,1 instead of concurrently hitting the same
            # PSUM bank (concurrent same-bank matmul writes are fatal)
            q_hi = qkp.tile([64, N], FP16, name="q_hi", tag="q_hi")
            k_hi = qkp.tile([64, N], FP16, name="k_hi", tag="k_hi")
            vt_all = vtp.tile([128, NMT * 512], BF16, name="vt_all", tag="vt_all")

            # ---------- phase 1: q chunk 0 + k chunk 0 only ----------
            with tc.tile_pool(name="p1", bufs=4, space="PSUM") as p1:
                ps = p1.tile([128, 512], F32, name="qproj", tag="p1")
                for c2 in range(2):
                    nc.tensor.matmul(ps[:], wqt[c2][:], xf[c2][:, 0:512],
                                     start=(c2 == 0), stop=(c2 == 1))
                nc.vector.tensor_scalar_add(q_all[:, 0:512], ps[:], bq_t[:])
                nc.vector.tensor_scalar_add(q_hi[:, 0:512], ps[64:128, :],
                                            bq_t[64:128, :])
                ps = p1.tile([128, 512], F32, name="kproj", tag="p1")
                for c2 in range(2):
                    nc.tensor.matmul(ps[:], wkt[c2][:], xf[c2][:, 0:512],
                                     start=(c2 == 0), stop=(c2 == 1))
                nc.vector.tensor_scalar_add(k_all[:, 0:512], ps[:], bk_t[:])
                nc.vector.tensor_scalar_add(k_hi[:, 0:512], ps[64:128, :],
                                            bk_t[64:128, :])

            # ---------- phase 2: attention + output projection ----------
            NP = NMT // 2
            G2 = ((0, 1), (2, 3))
            JCHUNKS = [(0, 512, G2), (512, 512, G2), (1024, 512, G2),
                       (1536, 512, G2), (2048, 256, ((0, 1, 2, 3),))]
            with tc.tile_pool(name="stp", bufs=1, space="PSUM") as stp, \
                 tc.tile_pool(name="xxp", bufs=1, space="PSUM") as xxp, \
                 tc.tile_pool(name="finp", bufs=1, space="PSUM") as finp:
                def emit_proj(joff_p, JW_p, r_p):
                    # output projection over all four heads of a finished
                    # chunk (deferred into the next chunk's loop so the PE
                    # never stalls on the finish chain at chunk boundaries)
                    for ct in range(2):
                        op_ps = finp.tile([128, 512], F32, name="op_ps",
                                          tag=f"f{ct}")
                        for h in range(NHG):
                            nc.tensor.matmul(
                                op_ps[:, 0:JW_p],
                                wpt[h][:, 128 * ct:128 * (ct + 1)],
                                r_p[h][:, 0:JW_p],
                                start=(h == 0), stop=(h == NHG - 1))
                        o_sb = workp.tile([128, 512], F32, name="o_sb",
                                          tag="o_sb")
                        nc.vector.tensor_copy(o_sb[:, 0:JW_p],
                                              op_ps[:, 0:JW_p])
                        nc.sync.dma_start(
                            out_d.ap()[128 * ct:128 * (ct + 1),
                                       joff_p:joff_p + JW_p],
                            o_sb[:, 0:JW_p])

                # q/k-projection schedules inside chunk 0 (mt -> chunk).
                QSCHED = {4: 1, 8: 2, 10: 3, 13: 4}
                KSCHED = {1: 1, 3: 2, 5: 3, 7: 4}

                pending = None     # (joff, JW, r_ts) of the previous chunk
                fin_prev = None    # previous pair's deferred finish closure
                pair_tasks = []
                for ci, (joff, JW, groups) in enumerate(JCHUNKS):
                    for gi, grp in enumerate(groups):
                        pair_tasks.append((ci, joff, JW, gi, grp,
                                           gi == len(groups) - 1))
                r_ts = None
                for ci, joff, JW, gi, grp, last_in_chunk in pair_tasks:
                    if True:
                        if gi == 0:
                            r_ts = [None] * NHG
                        G = len(grp)           # heads in this group
                        xxt = [xxp.tile([128, 512], F32, name=f"xx{j}",
                                        tag=f"xx{j}") for j in range(2)]
                        xx = {}
                        if G == 2:
                            # st/pe column and xx tile per head
                            colmap = {grp[0]: 0, grp[1]: 512}
                            for i, h in enumerate(grp):
                                xx[h] = xxt[i][:, 0:JW]
                        else:
                            # merged 4-head tail: concurrent pairs (0,1)
                            # then (2,3) land in different banks; bank
                            # pairing is (0,2) and (1,3)
                            colmap = {0: 0, 1: 512, 2: 256, 3: 768}
                            for h in grp:
                                xx[h] = xxt[h % 2][:, 256 * (h // 2):
                                                   256 * (h // 2) + JW]
                        pexp = [None] * NMT
                        tsum = [None] * NP
                        usum = [None] * 4

                        def emit_qk_exp(mt):
                            # heads 0,1 run concurrently in row groups 0,1
                            # (distinct banks); in the merged tail, heads
                            # 2,3 reuse row groups 0,1 (hi tiles) so they
                            # serialize behind 0,1 while filling the other
                            # half of each bank.
                            moff = 128 * mt
                            pe = pexpp.tile([128, 1024], BF16, name="pe",
                                            tag="pe", bufs=8)
                            st = stp.tile([128, 1024], F32, name="st",
                                          tag=f"st{mt % 2}")
                            for h in grp:
                                if G == 2 or h < 2:
                                    ks, qs, r0 = k_all, q_all, 32 * h
                                else:
                                    ks, qs, r0 = k_hi, q_hi, 32 * (h - 2)
                                nc.tensor.matmul(
                                    st[:, colmap[h]:colmap[h] + JW],
                                    ks[r0:r0 + 32, moff:moff + 128],
                                    qs[r0:r0 + 32, joff:joff + JW],
                                    start=True, stop=True,
                                    tile_position=(r0, 0))
                            if JW == 512 or G == 4:
                                nc.scalar.activation(pe[:, 0:1024],
                                                     st[:, 0:1024], AF.Exp)
                            else:
                                st_v = st.rearrange("p (a b) -> p a b",
                                                    b=512)[:, :, 0:JW]
                                pe_v = pe.rearrange("p (a b) -> p a b",
                                                    b=512)[:, :, 0:JW]
                                nc.scalar.activation(pe_v, st_v, AF.Exp)
                            pexp[mt] = pe

                        def emit_pv(mt, first=False):
                            # PV(1) executes first (psum reset); PV(0) joins
                            # late so the new pair's xx reset never waits on
                            # the previous pair's finish chain reading xx.
                            # start=True clears the whole bank's has_written
                            # bits, so only the first head per shared bank
                            # carries it (G==4).
                            pe = pexp[mt]
                            for h in grp:
                                nc.tensor.matmul(
                                    xx[h],
                                    vt_all[:, 512 * mt + 128 * h:
                                           512 * mt + 128 * (h + 1)],
                                    pe[:, colmap[h]:colmap[h] + JW],
                                    start=(first and (G == 2 or h < 2)),
                                    stop=(mt == NMT - 1),
                                    skip_group_check=True)

                        def emit_tree(k):
                            # u-level folds as soon as both t inputs are
                            # ready; each u-tile's ones-matmul column-sums
                            # follow immediately, spread through the loop
                            # to fill PE bubbles (the accumulating sums
                            # tiles are allocated at u0). t8 joins in the
                            # deferred finish.
                            if k % 2 == 1 and k < 8:
                                u = k // 2
                                t = workp.tile([128, 1024], BF16,
                                               name=f"u{u}", tag=f"u{u}",
                                               bufs=1)
                                nc.vector.tensor_tensor(
                                    t[:], tsum[k - 1][:], tsum[k][:], OP.add)
                                usum[u] = t

                        # PV schedule: PVs trail their exp by two steps so
                        # they never wait on the exp semaphore; the first
                        # PVs start later still, with a 2-per-step ramp.
                        FS = 4 if JW == 512 else 6
                        pv_order = [1, 2, 0] + list(range(3, NMT))
                        pv_sched = {}
                        done = 0
                        for _mt in range(FS, NMT):
                            target = min(NMT, _mt - 1)
                            n = min(2 if _mt > FS else 1,
                                    max(0, target - done))
                            if n:
                                pv_sched[_mt] = pv_order[done:done + n]
                                done += n
                        pv_left = pv_order[done:]

                        sums_hs = [None, None]
                        for mt in range(NMT):
                            if mt == 0 and fin_prev is not None:
                                # the previous pair's finish matmuls are
                                # ready now; emitting them ahead of ST(0)
                                # fills the PE's wait on the previous pair's
                                # trailing exp (WAR on the st buffer)
                                fin_prev()
                                fin_prev = None
                            emit_qk_exp(mt)
                            if ci == 0 and gi == 0:
                                # v projection for m-tile mt, one step ahead
                                # of its PV consumer
                                ps_v = finp.tile([128, 512], F32, name="vps",
                                                 tag=f"f{mt % 2}")
                                for c2 in range(2):
                                    nc.tensor.matmul(
                                        ps_v[:],
                                        xf[c2][:, 128 * mt:128 * (mt + 1)],
                                        wvt[c2][:],
                                        start=(c2 == 0), stop=(c2 == 1))
                                nc.vector.tensor_copy(
                                    vt_all[:, 512 * mt:512 * (mt + 1)],
                                    ps_v[:])
                                qk_c = [(QSCHED, wqt, q_all, q_hi, bq_t),
                                        (KSCHED, wkt, k_all, k_hi, bk_t)]
                                for sched, wt, dst, dhi, bias in qk_c:
                                    if mt not in sched:
                                        continue
                                    qo = 512 * sched[mt]
                                    qw = min(512, N - qo)
                                    ps_q = finp.tile([128, 512], F32,
                                                     name="qps",
                                                     tag=f"f{(mt + 1) % 2}")
                                    for c2 in range(2):
                                        nc.tensor.matmul(
                                            ps_q[:, 0:qw], wt[c2][:],
                                            xf[c2][:, qo:qo + qw],
                                            start=(c2 == 0), stop=(c2 == 1))
                                    nc.vector.tensor_scalar_add(
                                        dst[:, qo:qo + qw],
                                        ps_q[:, 0:qw], bias[:])
                                    nc.vector.tensor_scalar_add(
                                        dhi[:, qo:qo + qw],
                                        ps_q[64:128, 0:qw],
                                        bias[64:128, :])
                            for j in pv_sched.get(mt, ()):
                                emit_pv(j, first=(j == 1))
                            if mt == NMT - 1:
                                # u0..u3 cover m-tiles 0..15; start the
                                # denominator accumulation while the last
                                # exp is still in flight.
                                sw = JW if G == 2 else 512
                                for i in range(2):
                                    sums_hs[i] = finp.tile(
                                        [1, 512], F32, name="sums_h",
                                        tag=f"f{i % 2}")
                                    for uj in range(4):
                                        nc.tensor.matmul(
                                            sums_hs[i][:, 0:sw], ones_bf[:],
                                            usum[uj][:, 512 * i:512 * i + sw],
                                            start=(uj == 0), stop=False)
                            if mt == 3 and gi == 0 and pending is not None:
                                emit_proj(*pending)
                                pending = None
                            if mt % 2 == 1 and mt < NMT - 1:
                                k = mt // 2
                                t = workp.tile([128, 1024], BF16,
                                               name=f"t{k}", tag=f"t{k}",
                                               bufs=1)
                                nc.vector.tensor_tensor(
                                    t[:], pexp[mt - 1][:], pexp[mt][:],
                                    OP.add)
                                tsum[k] = t
                                emit_tree(k)
                        for j in pv_left:
                            emit_pv(j, first=(j == 1))

                        def make_finish(grp=grp, xx=xx, xxt_c=xxt,
                                        pexp=pexp, sums_hs=sums_hs,
                                        r_out=r_ts, JW=JW, G=G):
                            def fin():
                                # finish: t8, denominators, normalize,
                                # bias+relu. Only the t8 ones-matmul waits
                                # on the trailing exp.
                                sw = JW if G == 2 else 512
                                t8 = workp.tile([128, 1024], BF16,
                                                name="t8", tag="t8", bufs=1)
                                nc.vector.tensor_tensor(
                                    t8[:], pexp[16][:], pexp[17][:], OP.add)
                                for i in range(2):
                                    sums_h = sums_hs[i]
                                    nc.tensor.matmul(
                                        sums_h[:, 0:sw], ones_bf[:],
                                        t8[:, 512 * i:512 * i + sw],
                                        start=False, stop=True)
                                    s_inv = workp.tile([1, 512], F32,
                                                       name="s_inv",
                                                       tag="s_inv")
                                    nc.vector.reciprocal_approx_fast(
                                        s_inv[:, 0:sw], sums_h[:, 0:sw])
                                    inv_bc = workp.tile([128, 512], F32,
                                                        name="inv_bc",
                                                        tag="inv_bc")
                                    nc.gpsimd.partition_broadcast(
                                        inv_bc[:, 0:sw], s_inv[:, 0:sw])
                                    t_h = workp.tile([128, 512], F32,
                                                     name="t_h", tag="t_h")
                                    if G == 2:
                                        h = grp[i]
                                        nc.vector.tensor_tensor(
                                            t_h[:, 0:JW], xx[h],
                                            inv_bc[:, 0:JW], OP.mult)
                                        r_h = rp.tile([128, 512], FP16,
                                                      name=f"r{h}",
                                                      tag=f"r{h}")
                                        nc.vector.tensor_scalar(
                                            r_h[:, 0:JW], t_h[:, 0:JW],
                                            bv_t[h][:], 0.0, OP.add, OP.max)
                                        r_out[h] = r_h
                                    else:
                                        # bank i holds heads i (cols 0:256)
                                        # and i+2 (256:512), matching the
                                        # tree-tile column halves
                                        nc.vector.tensor_tensor(
                                            t_h[:, 0:512], xxt_c[i][:],
                                            inv_bc[:, 0:512], OP.mult)
                                        r_h = rp.tile([128, 512], FP16,
                                                      name=f"r{i}",
                                                      tag=f"r{i}")
                                        for hh in range(2):
                                            h = 2 * hh + i
                                            nc.vector.tensor_scalar(
                                                r_h[:, 256 * hh:
                                                    256 * hh + 256],
                                                t_h[:, 256 * hh:
                                                    256 * hh + 256],
                                                bv_t[h][:], 0.0,
                                                OP.add, OP.max)
                                            r_out[h] = r_h[:, 256 * hh:
                                                           256 * hh + 256]
                            return fin

                        fin_prev = make_finish()
                        if last_in_chunk:
                            pending = (joff, JW, r_ts)
                fin_prev()
                emit_proj(*pending)
    nc.compile()
    return nc


def _prep_inputs(x, wq, gq, bq, wk, gk, bk, wv, gv, bv, wp, gp, bp):
    """Fold BN scales into weights; build the 8 per-core input maps."""
    rs = np.float32(1.0 / np.sqrt(np.float32(1.0) + np.float32(EPS)))
    sq = (gq * rs).astype(np.float32)
    sk = (gk * rs).astype(np.float32)
    sv = (gv * rs).astype(np.float32)
    sp = (gp * rs).astype(np.float32)
    wq_f = (wq * sq[:, None]).astype(np.float16)
    wk_f = (wk * sk[:, None]).astype(np.float16)
    wv_f = (wv * sv[:, None]).astype(np.float16)
    wp_f = (wp * sp[:, None]).astype(np.float16)

    xf = np.ascontiguousarray(x.reshape(B, C, N).astype(np.float16))
    consts = np.zeros((128, 128 + J), dtype=np.float32)
    consts[:, 0:128] = 1.0
    import ml_dtypes
    ones_bf = np.ones((128, 1), dtype=ml_dtypes.bfloat16)
    in_maps = []
    for core in range(8):
        b, g = core // 2, core % 2
        qs = slice(128 * g, 128 * (g + 1))       # q/k rows for this head group
        vs = slice(512 * g, 512 * (g + 1))       # v rows / p cols for this group
        in_maps.append({
            "x_in": xf[b],
            "wqt": np.ascontiguousarray(wq_f[qs, :].T),
            "wkt": np.ascontiguousarray(wk_f[qs, :].T),
            "wvt": np.ascontiguousarray(wv_f[vs, :].T),
            "wpt": np.ascontiguousarray(wp_f[:, vs].T),
            "bq": np.ascontiguousarray(bq[qs].astype(np.float32)[:, None]),
            "bk": np.ascontiguousarray(bk[qs].astype(np.float32)[:, None]),
            "bv": np.ascontiguousarray(bv[vs].astype(np.float32)[:, None]),
            "consts": consts,
            "ones_bf": ones_bf,
        })
    return in_maps


def kernel(**inputs):
    if "nc" not in _CACHE:
        _CACHE["nc"] = _build_program()
    nc = _CACHE["nc"]

    in_maps = _prep_inputs(**{k: np.asarray(v) for k, v in inputs.items()})
    res = run_bass_kernel_spmd(nc, in_maps, list(range(8)))
    _CACHE["last_results"] = res

    bp = np.asarray(inputs["bp"]).astype(np.float32)
    out = np.empty((B, C, H, W), dtype=np.float32)
    for b in range(B):
        acc = res.results[2 * b]["outp"] + res.results[2 * b + 1]["outp"]
        acc = acc + bp[:, None]
        out[b] = acc.reshape(C, H, W)
    return out


# revision 35
# speedup vs baseline: 1.0650x; 1.0043x over previous
"""Trainium2 Bass kernel for the BN-attention module (nn_Attention).

Full inputs -> full output. Sharding: 8 cores = (batch b in 0..3) x
(head-group g in 0..1, 4 heads each). Each core computes its batch's
4-head attention and a partial output projection; the host sums the two
head-group partials per batch and adds the projection BN bias.

Numerics: BN scales are folded into the weights on the host. The Q/K
path (x, wq, wk, q, k) runs in fp16; exp input is fp32 PSUM and its
output bf16. All exp runs on the ScalarE: the exp instruction pace is
what the PE's ST stream is locked to (the two st PSUM buffers create a
write-after-read chain ST(mt+2) <- exp(mt)), so exp must stay on one
dedicated engine with deterministic FIFO latency -- offloading any of
it to the (queued) VectorE/GpSimd was measured to serialize the
pipeline.

Layout: attention is computed transposed, S^T = K^T Q with keys (m) on
partitions, so attn@V needs no transposes at all: V is produced
directly as vT[n,d] by the projection. Denominators are column sums:
exp tiles are pair-summed into t-tiles and pair-pair-summed into
u0..u3 on the VectorE (bf16 2x); the u-tiles and the trailing t8 are
column-summed by accumulating ones-matmuls (u0..u3 in one burst while
the last exp is in flight, filling the PE's wait at the pair
boundary). This keeps the denominator reduction off the ScalarE
entirely and replaces the deeper w-level VectorE folds of the previous
version with cheap PE work at the boundary. The finish (reciprocal on the
[1,JW] sums rows, GPSIMD partition_broadcast, normalize, fp16
bias+relu) is deferred into the next pair's loop, as is each chunk's
output projection, so the PE never stalls at pair/chunk boundaries.
Phase 1 is minimal (q/k chunk 0 only); the remaining q/k chunks and
all v projections are woven into chunk 0's m-tile loop, and input DMAs
are split across both HWDGE rings.
"""

import numpy as np

import concourse.bacc as bacc
import concourse.mybir as mybir
import concourse.tile as tile
from concourse.bass_utils import run_bass_kernel_spmd

# Problem dims (hardcoded per the spec)
B, C, H, W = 4, 256, 48, 48
N = H * W            # 2304
KD, NH, AR = 32, 8, 4
D = AR * KD          # 128 value dims per head
NHKD = NH * KD       # 256
DH = NH * D          # 1024
EPS = 1e-5

NHG = 4              # heads per core
J = 256              # (legacy) consts tile width
MT = 128             # m-tile (key tile)
NMT = N // MT        # 18

F32 = mybir.dt.float32
F32R = mybir.dt.float32r
BF16 = mybir.dt.bfloat16
FP16 = mybir.dt.float16
I16 = mybir.dt.int16
AF = mybir.ActivationFunctionType
OP = mybir.AluOpType

# Schraudolph bf16 fast-exp constants: bits = round(x*128/ln2 + 16256)
FE_A = float(128.0 / np.log(2.0))
FE_B = 16256.0

# chunks of 2304 by <=512 for the projection matmuls
CHUNKS_512 = [(off, min(512, N - off)) for off in range(0, N, 512)]

_CACHE = {}


def _build_program():
    nc = bacc.Bacc("TRN2", target_bir_lowering=False, debug=False)

    x_in = nc.dram_tensor("x_in", [C, N], FP16, kind="ExternalInput")
    wqt_d = nc.dram_tensor("wqt", [C, 128], FP16, kind="ExternalInput")
    wkt_d = nc.dram_tensor("wkt", [C, 128], FP16, kind="ExternalInput")
    wvt_d = nc.dram_tensor("wvt", [C, 512], FP16, kind="ExternalInput")
    wpt_d = nc.dram_tensor("wpt", [512, C], FP16, kind="ExternalInput")
    bq_d = nc.dram_tensor("bq", [128, 1], F32, kind="ExternalInput")
    bk_d = nc.dram_tensor("bk", [128, 1], F32, kind="ExternalInput")
    bv_d = nc.dram_tensor("bv", [512, 1], F32, kind="ExternalInput")
    consts_d = nc.dram_tensor("consts", [128, 128 + J], F32R, kind="ExternalInput")
    ones_bf_d = nc.dram_tensor("ones_bf", [128, 1], BF16, kind="ExternalInput")
    out_d = nc.dram_tensor("outp", [C, N], F32, kind="ExternalOutput")

    with tile.TileContext(nc) as tc:
        with nc.allow_low_precision(reason="16-bit matmul rounding is intentional"), \
             tc.tile_pool(name="const", bufs=1) as constp, \
             tc.tile_pool(name="qk", bufs=1) as qkp, \
             tc.tile_pool(name="vt", bufs=1) as vtp, \
             tc.tile_pool(name="pexp", bufs=1) as pexpp, \
             tc.tile_pool(name="rp", bufs=1) as rp, \
             tc.tile_pool(name="work", bufs=2) as workp:

            # ---------- constants / inputs ----------
            xf = [constp.tile([128, N], FP16, name=f"xf{c2}", tag=f"xf{c2}")
                  for c2 in range(2)]
            wqt, wkt, wvt = [], [], []
            nc.sync.dma_start(xf[0][:, 0:512], x_in.ap()[0:128, 0:512])
            for c2 in range(2):
                sl = slice(128 * c2, 128 * (c2 + 1))
                t = constp.tile([128, 128], FP16, name=f"wqt{c2}", tag=f"wqt{c2}")
                nc.scalar.dma_start(t[:], wqt_d.ap()[sl, :])
                wqt.append(t)
                t = constp.tile([128, 128], FP16, name=f"wkt{c2}", tag=f"wkt{c2}")
                nc.scalar.dma_start(t[:], wkt_d.ap()[sl, :])
                wkt.append(t)
            nc.sync.dma_start(xf[1][:, 0:512], x_in.ap()[128:256, 0:512])
            bq_t = constp.tile([128, 1], F32, name="bq_t", tag="bq_t")
            nc.scalar.dma_start(bq_t[:], bq_d.ap())
            bk_t = constp.tile([128, 1], F32, name="bk_t", tag="bk_t")
            nc.scalar.dma_start(bk_t[:], bk_d.ap())
            for c2 in range(2):
                t = constp.tile([128, 512], FP16, name=f"wvt{c2}",
                                tag=f"wvt{c2}")
                eng = nc.sync if c2 == 0 else nc.scalar
                eng.dma_start(t[:], wvt_d.ap()[128 * c2:128 * (c2 + 1), :])
                wvt.append(t)
            for off, w in CHUNKS_512:
                if off == 0:
                    continue
                nc.sync.dma_start(xf[0][:, off:off + w],
                                  x_in.ap()[0:128, off:off + w])
                nc.scalar.dma_start(xf[1][:, off:off + w],
                                    x_in.ap()[128:256, off:off + w])
            wpt = []
            for h in range(NHG):
                t = constp.tile([128, C], FP16, name=f"wpt{h}", tag=f"wpt{h}")
                eng = nc.sync if h % 2 == 0 else nc.scalar
                eng.dma_start(t[:], wpt_d.ap()[128 * h:128 * (h + 1), :])
                wpt.append(t)
            bv_t = []
            for h in range(NHG):
                t = constp.tile([128, 1], F32, name=f"bv{h}", tag=f"bv{h}")
                nc.sync.dma_start(t[:], bv_d.ap()[128 * h:128 * (h + 1), :])
                bv_t.append(t)
            ones_bf = constp.tile([128, 1], BF16, name="ones_bf", tag="ones_bf")
            nc.sync.dma_start(ones_bf[:], ones_bf_d.ap())

            q_all = qkp.tile([128, N], FP16, name="q_all", tag="q_all")
            k_all = qkp.tile([128, N], FP16, name="k_all", tag="k_all")
            # heads 2,3 replicated on partitions 0:64: the merged tail
            # chunk runs them in PE row groups 0,1 so they serialize
            # behind heads 0,1 instead of concurrently hitting the same
            # PSUM bank (concurrent same-bank matmul writes are fatal)
            q_hi = qkp.tile([64, N], FP16, name="q_hi", tag="q_hi")
            k_hi = qkp.tile([64, N], FP16, name="k_hi", tag="k_hi")
            vt_all = vtp.tile([128, NMT * 512], BF16, name="vt_all", tag="vt_all")

            # ---------- phase 1: q chunk 0 + k chunk 0 only ----------
            with tc.tile_pool(name="p1", bufs=4, space="PSUM") as p1:
                ps = p1.tile([128, 512], F32, name="qproj", tag="p1")
                for c2 in range(2):
                    nc.tensor.matmul(ps[:], wqt[c2][:], xf[c2][:, 0:512],
                                     start=(c2 == 0), stop=(c2 == 1))
                nc.vector.tensor_scalar_add(q_all[:, 0:512], ps[:], bq_t[:])
                nc.vector.tensor_scalar_add(q_hi[:, 0:512], ps[64:128, :],
                                            bq_t[64:128, :])
                ps = p1.tile([128, 512], F32, name="kproj", tag="p1")
                for c2 in range(2):
                    nc.tensor.matmul(ps[:], wkt[c2][:], xf[c2][:, 0:512],
                                     start=(c2 == 0), stop=(c2 == 1))
                nc.vector.tensor_scalar_add(k_all[:, 0:512], ps[:], bk_t[:])
                nc.vector.tensor_scalar_add(k_hi[:, 0:512], ps[64:128, :],
                                            bk_t[64:128, :])

            # ---------- phase 2: attention + output projection ----------
            NP = NMT // 2
            G2 = ((0, 1), (2, 3))
            JCHUNKS = [(0, 512, G2), (512, 512, G2), (1024, 512, G2),
                       (1536, 512, G2), (2048, 256, ((0, 1, 2, 3),))]
            with tc.tile_pool(name="stp", bufs=1, space="PSUM") as stp, \
                 tc.tile_pool(name="xxp", bufs=1, space="PSUM") as xxp, \
                 tc.tile_pool(name="finp", bufs=1, space="PSUM") as finp:
                def emit_proj(joff_p, JW_p, r_p):
                    # output projection over all four heads of a finished
                    # chunk (deferred into the next chunk's loop so the PE
                    # never stalls on the finish chain at chunk boundaries)
                    for ct in range(2):
                        op_ps = finp.tile([128, 512], F32, name="op_ps",
                                          tag=f"f{ct}")
                        for h in range(NHG):
                            nc.tensor.matmul(
                                op_ps[:, 0:JW_p],
                                wpt[h][:, 128 * ct:128 * (ct + 1)],
                                r_p[h][:, 0:JW_p],
                                start=(h == 0), stop=(h == NHG - 1))
                        o_sb = workp.tile([128, 512], F32, name="o_sb",
                                          tag="o_sb")
                        nc.vector.tensor_copy(o_sb[:, 0:JW_p],
                                              op_ps[:, 0:JW_p])
                        nc.sync.dma_start(
                            out_d.ap()[128 * ct:128 * (ct + 1),
                                       joff_p:joff_p + JW_p],
                            o_sb[:, 0:JW_p])

                # q/k-projection schedules inside chunk 0 (mt -> chunk).
                QSCHED = {4: 1, 8: 2, 10: 3, 13: 4}
                KSCHED = {1: 1, 3: 2, 5: 3, 7: 4}

                pending = None     # (joff, JW, r_ts) of the previous chunk
                fin_prev = None    # previous pair's deferred finish closure
                pair_tasks = []
                for ci, (joff, JW, groups) in enumerate(JCHUNKS):
                    for gi, grp in enumerate(groups):
                        pair_tasks.append((ci, joff, JW, gi, grp,
                                           gi == len(groups) - 1))
                r_ts = None
                for ci, joff, JW, gi, grp, last_in_chunk in pair_tasks:
                    if True:
                        if gi == 0:
                            r_ts = [None] * NHG
                        G = len(grp)           # heads in this group
                        xxt = [xxp.tile([128, 512], F32, name=f"xx{j}",
                                        tag=f"xx{j}") for j in range(2)]
                        xx = {}
                        if G == 2:
                            # st/pe column and xx tile per head
                            colmap = {grp[0]: 0, grp[1]: 512}
                            for i, h in enumerate(grp):
                                xx[h] = xxt[i][:, 0:JW]
                        else:
                            # merged 4-head tail: concurrent pairs (0,1)
                            # then (2,3) land in different banks; bank
                            # pairing is (0,2) and (1,3)
                            colmap = {0: 0, 1: 512, 2: 256, 3: 768}
                            for h in grp:
                                xx[h] = xxt[h % 2][:, 256 * (h // 2):
                                                   256 * (h // 2) + JW]
                        pexp = [None] * NMT
                        tsum = [None] * NP
                        usum = [None] * 4

                        def emit_qk_exp(mt):
                            # heads 0,1 run concurrently in row groups 0,1
                            # (distinct banks); in the merged tail, heads
                            # 2,3 reuse row groups 0,1 (hi tiles) so they
                            # serialize behind 0,1 while filling the other
                            # half of each bank.
                            moff = 128 * mt
                            pe = pexpp.tile([128, 1024], BF16, name="pe",
                                            tag="pe", bufs=8)
                            st = stp.tile([128, 1024], F32, name="st",
                                          tag=f"st{mt % 2}")
                            for h in grp:
                                if G == 2 or h < 2:
                                    ks, qs, r0 = k_all, q_all, 32 * h
                                else:
                                    ks, qs, r0 = k_hi, q_hi, 32 * (h - 2)
                                nc.tensor.matmul(
                                    st[:, colmap[h]:colmap[h] + JW],
                                    ks[r0:r0 + 32, moff:moff + 128],
                                    qs[r0:r0 + 32, joff:joff + JW],
                                    start=True, stop=True,
                                    tile_position=(r0, 0))
                            if JW == 512 or G == 4:
                                nc.scalar.activation(pe[:, 0:1024],
                                                     st[:, 0:1024], AF.Exp)
                            else:
                                st_v = st.rearrange("p (a b) -> p a b",
                                                    b=512)[:, :, 0:JW]
                                pe_v = pe.rearrange("p (a b) -> p a b",
                                                    b=512)[:, :, 0:JW]
                                nc.scalar.activation(pe_v, st_v, AF.Exp)
                            pexp[mt] = pe

                        def emit_pv(mt, first=False):
                            # PV(1) executes first (psum reset); PV(0) joins
                            # late so the new pair's xx reset never waits on
                            # the previous pair's finish chain reading xx.
                            # start=True clears the whole bank's has_written
                            # bits, so only the first head per shared bank
                            # carries it (G==4).
                            pe = pexp[mt]
                            for h in grp:
                                nc.tensor.matmul(
                                    xx[h],
                                    vt_all[:, 512 * mt + 128 * h:
                                           512 * mt + 128 * (h + 1)],
                                    pe[:, colmap[h]:colmap[h] + JW],
                                    start=(first and (G == 2 or h < 2)),
                                    stop=(mt == NMT - 1),
                                    skip_group_check=True)

                        def emit_tree(k):
                            # u-level folds as soon as both t inputs are
                            # ready; each u-tile's ones-matmul column-sums
                            # follow immediately, spread through the loop
                            # to fill PE bubbles (the accumulating sums
                            # tiles are allocated at u0). t8 joins in the
                            # deferred finish.
                            if k % 2 == 1 and k < 8:
                                u = k // 2
                                t = workp.tile([128, 1024], BF16,
                                               name=f"u{u}", tag=f"u{u}",
                                               bufs=1)
                                nc.vector.tensor_tensor(
                                    t[:], tsum[k - 1][:], tsum[k][:], OP.add)
                                usum[u] = t

                        # PV schedule: PVs trail their exp by two steps so
                        # they never wait on the exp semaphore; the first
                        # PVs start later still, with a 2-per-step ramp.
                        FS = 4 if JW == 512 else 6
                        pv_order = [1, 2, 0] + list(range(3, NMT))
                        pv_sched = {}
                        done = 0
                        for _mt in range(FS, NMT):
                            target = min(NMT, _mt - 1)
                            n = min(2 if _mt > FS else 1,
                                    max(0, target - done))
                            if n:
                                pv_sched[_mt] = pv_order[done:done + n]
                                done += n
                        pv_left = pv_order[done:]

                        sums_hs = [None, None]
                        for mt in range(NMT):
                            if mt == 0 and fin_prev is not None:
                                # t8 add (VectorE only) first: by the time
                                # the PE-side finish runs at mt 1, t8 is
                                # done and the sums stop-matmuls don't
                                # stall ST(0)/ST(1) behind DVE latency
                                fin_prev[0]()
                            if mt == 1 and fin_prev is not None:
                                fin_prev[1]()
                                fin_prev = None
                            emit_qk_exp(mt)
                            if ci == 0 and gi == 0:
                                # v projection for m-tile mt, one step ahead
                                # of its PV consumer
                                ps_v = finp.tile([128, 512], F32, name="vps",
                                                 tag=f"f{mt % 2}")
                                for c2 in range(2):
                                    nc.tensor.matmul(
                                        ps_v[:],
                                        xf[c2][:, 128 * mt:128 * (mt + 1)],
                                        wvt[c2][:],
                                        start=(c2 == 0), stop=(c2 == 1))
                                nc.vector.tensor_copy(
                                    vt_all[:, 512 * mt:512 * (mt + 1)],
                                    ps_v[:])
                                qk_c = [(QSCHED, wqt, q_all, q_hi, bq_t),
                                        (KSCHED, wkt, k_all, k_hi, bk_t)]
                                for sched, wt, dst, dhi, bias in qk_c:
                                    if mt not in sched:
                                        continue
                                    qo = 512 * sched[mt]
                                    qw = min(512, N - qo)
                                    ps_q = finp.tile([128, 512], F32,
                                                     name="qps",
                                                     tag=f"f{(mt + 1) % 2}")
                                    for c2 in range(2):
                                        nc.tensor.matmul(
                                            ps_q[:, 0:qw], wt[c2][:],
                                            xf[c2][:, qo:qo + qw],
                                            start=(c2 == 0), stop=(c2 == 1))
                                    nc.vector.tensor_scalar_add(
                                        dst[:, qo:qo + qw],
                                        ps_q[:, 0:qw], bias[:])
                                    nc.vector.tensor_scalar_add(
                                        dhi[:, qo:qo + qw],
                                        ps_q[64:128, 0:qw],
                                        bias[64:128, :])
                            for j in pv_sched.get(mt, ()):
                                emit_pv(j, first=(j == 1))
                            if mt == NMT - 1:
                                # u0..u3 cover m-tiles 0..15; start the
                                # denominator accumulation while the last
                                # exp is still in flight.
                                sw = JW if G == 2 else 512
                                for i in range(2):
                                    sums_hs[i] = finp.tile(
                                        [1, 512], F32, name="sums_h",
                                        tag=f"f{i % 2}")
                                    for uj in range(4):
                                        nc.tensor.matmul(
                                            sums_hs[i][:, 0:sw], ones_bf[:],
                                            usum[uj][:, 512 * i:512 * i + sw],
                                            start=(uj == 0), stop=False)
                            if mt == 3 and gi == 0 and pending is not None:
                                emit_proj(*pending)
                                pending = None
                            if mt % 2 == 1 and mt < NMT - 1:
                                k = mt // 2
                                t = workp.tile([128, 1024], BF16,
                                               name=f"t{k}", tag=f"t{k}",
                                               bufs=1)
                                nc.vector.tensor_tensor(
                                    t[:], pexp[mt - 1][:], pexp[mt][:],
                                    OP.add)
                                tsum[k] = t
                                emit_tree(k)
                        for j in pv_left:
                            emit_pv(j, first=(j == 1))

                        def make_finish(grp=grp, xx=xx, xxt_c=xxt,
                                        pexp=pexp, sums_hs=sums_hs,
                                        r_out=r_ts, JW=JW, G=G):
                            cell = [None]

                            def fin_a():
                                # t8 add only (VectorE; no PE instruction)
                                t8 = workp.tile([128, 1024], BF16,
                                                name="t8", tag="t8", bufs=1)
                                nc.vector.tensor_tensor(
                                    t8[:], pexp[16][:], pexp[17][:], OP.add)
                                cell[0] = t8

                            def fin_b():
                                # denominators, normalize, bias+relu.
                                # Phases interleave the halves so the GPS
                                # broadcast of half 0 overlaps the DVE
                                # reciprocal of half 1.
                                sw = JW if G == 2 else 512
                                t8 = cell[0]
                                s_invs = [None, None]
                                bcs = [None, None]
                                for i in range(2):
                                    nc.tensor.matmul(
                                        sums_hs[i][:, 0:sw], ones_bf[:],
                                        t8[:, 512 * i:512 * i + sw],
                                        start=False, stop=True)
                                for i in range(2):
                                    s_inv = workp.tile([1, 512], F32,
                                                       name="s_inv",
                                                       tag=f"s_inv{i}")
                                    nc.vector.reciprocal_approx_fast(
                                        s_inv[:, 0:sw],
                                        sums_hs[i][:, 0:sw])
                                    s_invs[i] = s_inv
                                for i in range(2):
                                    bc = workp.tile([128, 512], F32,
                                                    name="inv_bc",
                                                    tag=f"inv_bc{i}")
                                    nc.gpsimd.partition_broadcast(
                                        bc[:, 0:sw], s_invs[i][:, 0:sw])
                                    bcs[i] = bc
                                for i in range(2):
                                    t_h = workp.tile([128, 512], F32,
                                                     name="t_h", tag="t_h")
                                    if G == 2:
                                        h = grp[i]
                                        nc.vector.tensor_tensor(
                                            t_h[:, 0:JW], xx[h],
                                            bcs[i][:, 0:JW], OP.mult)
                                        r_h = rp.tile([128, 512], FP16,
                                                      name=f"r{h}",
                                                      tag=f"r{h}")
                                        nc.vector.tensor_scalar(
                                            r_h[:, 0:JW], t_h[:, 0:JW],
                                            bv_t[h][:], 0.0, OP.add, OP.max)
                                        r_out[h] = r_h
                                    else:
                                        # bank i holds heads i (cols 0:256)
                                        # and i+2 (256:512)
                                        nc.vector.tensor_tensor(
                                            t_h[:, 0:512], xxt_c[i][:],
                                            bcs[i][:, 0:512], OP.mult)
                                        r_h = rp.tile([128, 512], FP16,
                                                      name=f"r{i}",
                                                      tag=f"r{i}")
                                        for hh in range(2):
                                            h = 2 * hh + i
                                            nc.vector.tensor_scalar(
                                                r_h[:, 256 * hh:
                                                    256 * hh + 256],
                                                t_h[:, 256 * hh:
                                                    256 * hh + 256],
                                                bv_t[h][:], 0.0,
                                                OP.add, OP.max)
                                            r_out[h] = r_h[:, 256 * hh:
                                                           256 * hh + 256]
                            return (fin_a, fin_b)

                        fin_prev = make_finish()
                        if last_in_chunk:
                            pending = (joff, JW, r_ts)
                fin_prev[0]()
                fin_prev[1]()
                emit_proj(*pending)
    nc.compile()
    return nc


def _prep_inputs(x, wq, gq, bq, wk, gk, bk, wv, gv, bv, wp, gp, bp):
    """Fold BN scales into weights; build the 8 per-core input maps."""
    rs = np.float32(1.0 / np.sqrt(np.float32(1.0) + np.float32(EPS)))
    sq = (gq * rs).astype(np.float32)
    sk = (gk * rs).astype(np.float32)
    sv = (gv * rs).astype(np.float32)
    sp = (gp * rs).astype(np.float32)
    wq_f = (wq * sq[:, None]).astype(np.float16)
    wk_f = (wk * sk[:, None]).astype(np.float16)
    wv_f = (wv * sv[:, None]).astype(np.float16)
    wp_f = (wp * sp[:, None]).astype(np.float16)

    xf = np.ascontiguousarray(x.reshape(B, C, N).astype(np.float16))
    consts = np.zeros((128, 128 + J), dtype=np.float32)
    consts[:, 0:128] = 1.0
    import ml_dtypes
    ones_bf = np.ones((128, 1), dtype=ml_dtypes.bfloat16)
    in_maps = []
    for core in range(8):
        b, g = core // 2, core % 2
        qs = slice(128 * g, 128 * (g + 1))       # q/k rows for this head group
        vs = slice(512 * g, 512 * (g + 1))       # v rows / p cols for this group
        in_maps.append({
            "x_in": xf[b],
            "wqt": np.ascontiguousarray(wq_f[qs, :].T),
            "wkt": np.ascontiguousarray(wk_f[qs, :].T),
            "wvt": np.ascontiguousarray(wv_f[vs, :].T),
            "wpt": np.ascontiguousarray(wp_f[:, vs].T),
            "bq": np.ascontiguousarray(bq[qs].astype(np.float32)[:, None]),
            "bk": np.ascontiguousarray(bk[qs].astype(np.float32)[:, None]),
            "bv": np.ascontiguousarray(bv[vs].astype(np.float32)[:, None]),
            "consts": consts,
            "ones_bf": ones_bf,
        })
    return in_maps


def kernel(**inputs):
    if "nc" not in _CACHE:
        _CACHE["nc"] = _build_program()
    nc = _CACHE["nc"]

    in_maps = _prep_inputs(**{k: np.asarray(v) for k, v in inputs.items()})
    res = run_bass_kernel_spmd(nc, in_maps, list(range(8)))
    _CACHE["last_results"] = res

    bp = np.asarray(inputs["bp"]).astype(np.float32)
    out = np.empty((B, C, H, W), dtype=np.float32)
    for b in range(B):
        acc = res.results[2 * b]["outp"] + res.results[2 * b + 1]["outp"]
        acc = acc + bp[:, None]
        out[b] = acc.reshape(C, H, W)
    return out


# revision 37
# speedup vs baseline: 1.0685x; 1.0032x over previous
"""Trainium2 Bass kernel for the BN-attention module (nn_Attention).

Full inputs -> full output. Sharding: 8 cores = (batch b in 0..3) x
(head-group g in 0..1, 4 heads each). Each core computes its batch's
4-head attention and a partial output projection; the host sums the two
head-group partials per batch and adds the projection BN bias.

Numerics: BN scales are folded into the weights on the host. The Q/K
path (x, wq, wk, q, k) runs in fp16; exp input is fp32 PSUM and its
output bf16. All exp runs on the ScalarE: the exp instruction pace is
what the PE's ST stream is locked to (the two st PSUM buffers create a
write-after-read chain ST(mt+2) <- exp(mt)), so exp must stay on one
dedicated engine with deterministic FIFO latency -- offloading any of
it to the (queued) VectorE/GpSimd was measured to serialize the
pipeline.

Layout: attention is computed transposed, S^T = K^T Q with keys (m) on
partitions, so attn@V needs no transposes at all: V is produced
directly as vT[n,d] by the projection. Denominators are column sums:
exp tiles are pair-summed into t-tiles and pair-pair-summed into
u0..u3 on the VectorE (bf16 2x); the u-tiles and the trailing t8 are
column-summed by accumulating ones-matmuls (u0..u3 in one burst while
the last exp is in flight, filling the PE's wait at the pair
boundary). This keeps the denominator reduction off the ScalarE
entirely and replaces the deeper w-level VectorE folds of the previous
version with cheap PE work at the boundary. The finish (reciprocal on the
[1,JW] sums rows, GPSIMD partition_broadcast, normalize, fp16
bias+relu) is deferred into the next pair's loop, as is each chunk's
output projection, so the PE never stalls at pair/chunk boundaries.
Phase 1 is minimal (q/k chunk 0 only); the remaining q/k chunks and
all v projections are woven into chunk 0's m-tile loop, and input DMAs
are split across both HWDGE rings.
"""

import numpy as np

import concourse.bacc as bacc
import concourse.mybir as mybir
import concourse.tile as tile
from concourse.bass_utils import run_bass_kernel_spmd

# Problem dims (hardcoded per the spec)
B, C, H, W = 4, 256, 48, 48
N = H * W            # 2304
KD, NH, AR = 32, 8, 4
D = AR * KD          # 128 value dims per head
NHKD = NH * KD       # 256
DH = NH * D          # 1024
EPS = 1e-5

NHG = 4              # heads per core
J = 256              # (legacy) consts tile width
MT = 128             # m-tile (key tile)
NMT = N // MT        # 18

F32 = mybir.dt.float32
F32R = mybir.dt.float32r
BF16 = mybir.dt.bfloat16
FP16 = mybir.dt.float16
I16 = mybir.dt.int16
AF = mybir.ActivationFunctionType
OP = mybir.AluOpType

# Schraudolph bf16 fast-exp constants: bits = round(x*128/ln2 + 16256)
FE_A = float(128.0 / np.log(2.0))
FE_B = 16256.0

# chunks of 2304 by <=512 for the projection matmuls
CHUNKS_512 = [(off, min(512, N - off)) for off in range(0, N, 512)]

_CACHE = {}


def _build_program():
    nc = bacc.Bacc("TRN2", target_bir_lowering=False, debug=False)

    x_in = nc.dram_tensor("x_in", [C, N], FP16, kind="ExternalInput")
    wqt_d = nc.dram_tensor("wqt", [C, 128], FP16, kind="ExternalInput")
    wkt_d = nc.dram_tensor("wkt", [C, 128], FP16, kind="ExternalInput")
    wvt_d = nc.dram_tensor("wvt", [C, 512], FP16, kind="ExternalInput")
    wpt_d = nc.dram_tensor("wpt", [512, C], FP16, kind="ExternalInput")
    bq_d = nc.dram_tensor("bq", [128, 1], F32, kind="ExternalInput")
    bk_d = nc.dram_tensor("bk", [128, 1], F32, kind="ExternalInput")
    bv_d = nc.dram_tensor("bv", [512, 1], F32, kind="ExternalInput")
    consts_d = nc.dram_tensor("consts", [128, 128 + J], F32R, kind="ExternalInput")
    ones_bf_d = nc.dram_tensor("ones_bf", [128, 1], BF16, kind="ExternalInput")
    out_d = nc.dram_tensor("outp", [C, N], F32, kind="ExternalOutput")

    with tile.TileContext(nc) as tc:
        with nc.allow_low_precision(reason="16-bit matmul rounding is intentional"), \
             tc.tile_pool(name="const", bufs=1) as constp, \
             tc.tile_pool(name="qk", bufs=1) as qkp, \
             tc.tile_pool(name="vt", bufs=1) as vtp, \
             tc.tile_pool(name="pexp", bufs=1) as pexpp, \
             tc.tile_pool(name="rp", bufs=1) as rp, \
             tc.tile_pool(name="work", bufs=2) as workp:

            # ---------- constants / inputs ----------
            xf = [constp.tile([128, N], FP16, name=f"xf{c2}", tag=f"xf{c2}")
                  for c2 in range(2)]
            wqt, wkt, wvt = [], [], []
            nc.sync.dma_start(xf[0][:, 0:512], x_in.ap()[0:128, 0:512])
            for c2 in range(2):
                sl = slice(128 * c2, 128 * (c2 + 1))
                t = constp.tile([128, 128], FP16, name=f"wqt{c2}", tag=f"wqt{c2}")
                nc.scalar.dma_start(t[:], wqt_d.ap()[sl, :])
                wqt.append(t)
                t = constp.tile([128, 128], FP16, name=f"wkt{c2}", tag=f"wkt{c2}")
                nc.scalar.dma_start(t[:], wkt_d.ap()[sl, :])
                wkt.append(t)
            nc.sync.dma_start(xf[1][:, 0:512], x_in.ap()[128:256, 0:512])
            bq_t = constp.tile([128, 1], F32, name="bq_t", tag="bq_t")
            nc.scalar.dma_start(bq_t[:], bq_d.ap())
            bk_t = constp.tile([128, 1], F32, name="bk_t", tag="bk_t")
            nc.scalar.dma_start(bk_t[:], bk_d.ap())
            for c2 in range(2):
                t = constp.tile([128, 512], FP16, name=f"wvt{c2}",
                                tag=f"wvt{c2}")
                eng = nc.sync if c2 == 0 else nc.scalar
                eng.dma_start(t[:], wvt_d.ap()[128 * c2:128 * (c2 + 1), :])
                wvt.append(t)
            for off, w in CHUNKS_512:
                if off == 0:
                    continue
                nc.sync.dma_start(xf[0][:, off:off + w],
                                  x_in.ap()[0:128, off:off + w])
                nc.scalar.dma_start(xf[1][:, off:off + w],
                                    x_in.ap()[128:256, off:off + w])
            wpt = []
            for h in range(NHG):
                t = constp.tile([128, C], FP16, name=f"wpt{h}", tag=f"wpt{h}")
                eng = nc.sync if h % 2 == 0 else nc.scalar
                eng.dma_start(t[:], wpt_d.ap()[128 * h:128 * (h + 1), :])
                wpt.append(t)
            bv_t = []
            for h in range(NHG):
                t = constp.tile([128, 1], F32, name=f"bv{h}", tag=f"bv{h}")
                nc.sync.dma_start(t[:], bv_d.ap()[128 * h:128 * (h + 1), :])
                bv_t.append(t)
            ones_bf = constp.tile([128, 1], BF16, name="ones_bf", tag="ones_bf")
            nc.sync.dma_start(ones_bf[:], ones_bf_d.ap())

            q_all = qkp.tile([128, N], FP16, name="q_all", tag="q_all")
            k_all = qkp.tile([128, N], FP16, name="k_all", tag="k_all")
            # heads 2,3 replicated on partitions 0:64: the merged tail
            # chunk runs them in PE row groups 0,1 so they serialize
            # behind heads 0,1 instead of concurrently hitting the same
            # PSUM bank (concurrent same-bank matmul writes are fatal)
            q_hi = qkp.tile([64, N], FP16, name="q_hi", tag="q_hi")
            k_hi = qkp.tile([64, N], FP16, name="k_hi", tag="k_hi")
            vt_all = vtp.tile([128, NMT * 512], BF16, name="vt_all", tag="vt_all")

            # ---------- phase 1: q chunk 0 + k chunk 0 only ----------
            with tc.tile_pool(name="p1", bufs=4, space="PSUM") as p1:
                ps = p1.tile([128, 512], F32, name="qproj", tag="p1")
                for c2 in range(2):
                    nc.tensor.matmul(ps[:], wqt[c2][:], xf[c2][:, 0:512],
                                     start=(c2 == 0), stop=(c2 == 1))
                nc.vector.tensor_scalar_add(q_all[:, 0:512], ps[:], bq_t[:])
                nc.vector.tensor_scalar_add(q_hi[:, 0:512], ps[64:128, :],
                                            bq_t[64:128, :])
                ps = p1.tile([128, 512], F32, name="kproj", tag="p1")
                for c2 in range(2):
                    nc.tensor.matmul(ps[:], wkt[c2][:], xf[c2][:, 0:512],
                                     start=(c2 == 0), stop=(c2 == 1))
                nc.vector.tensor_scalar_add(k_all[:, 0:512], ps[:], bk_t[:])
                nc.vector.tensor_scalar_add(k_hi[:, 0:512], ps[64:128, :],
                                            bk_t[64:128, :])

            # ---------- phase 2: attention + output projection ----------
            NP = NMT // 2
            G2 = ((0, 1), (2, 3))
            JCHUNKS = [(0, 512, G2), (512, 512, G2), (1024, 512, G2),
                       (1536, 512, G2), (2048, 256, ((0, 1, 2, 3),))]
            with tc.tile_pool(name="stp", bufs=1, space="PSUM") as stp, \
                 tc.tile_pool(name="xxp", bufs=1, space="PSUM") as xxp, \
                 tc.tile_pool(name="finp", bufs=1, space="PSUM") as finp:
                def emit_proj(joff_p, JW_p, r_p):
                    # output projection over all four heads of a finished
                    # chunk (deferred into the next chunk's loop so the PE
                    # never stalls on the finish chain at chunk boundaries)
                    for ct in range(2):
                        op_ps = finp.tile([128, 512], F32, name="op_ps",
                                          tag=f"f{ct}")
                        for h in range(NHG):
                            nc.tensor.matmul(
                                op_ps[:, 0:JW_p],
                                wpt[h][:, 128 * ct:128 * (ct + 1)],
                                r_p[h][:, 0:JW_p],
                                start=(h == 0), stop=(h == NHG - 1))
                        o_sb = workp.tile([128, 512], F32, name="o_sb",
                                          tag="o_sb")
                        nc.vector.tensor_copy(o_sb[:, 0:JW_p],
                                              op_ps[:, 0:JW_p])
                        nc.sync.dma_start(
                            out_d.ap()[128 * ct:128 * (ct + 1),
                                       joff_p:joff_p + JW_p],
                            o_sb[:, 0:JW_p])

                # q/k-projection schedules inside chunk 0 (mt -> chunk).
                QSCHED = {4: 1, 8: 2, 10: 3, 13: 4}
                KSCHED = {1: 1, 3: 2, 5: 3, 7: 4}

                pending = None     # (joff, JW, r_ts) of the previous chunk
                fin_prev = None    # previous pair's deferred finish closure
                carry = None       # previous pair's deferred boundary burst
                pair_tasks = []
                for ci, (joff, JW, groups) in enumerate(JCHUNKS):
                    for gi, grp in enumerate(groups):
                        pair_tasks.append((ci, joff, JW, gi, grp,
                                           gi == len(groups) - 1))
                r_ts = None
                for ti, (ci, joff, JW, gi, grp,
                         last_in_chunk) in enumerate(pair_tasks):
                    is_last = ti == len(pair_tasks) - 1
                    if True:
                        if gi == 0:
                            r_ts = [None] * NHG
                        G = len(grp)           # heads in this group
                        xxt = [xxp.tile([128, 512], F32, name=f"xx{j}",
                                        tag=f"xx{j}") for j in range(2)]
                        xx = {}
                        if G == 2:
                            # st/pe column and xx tile per head
                            colmap = {grp[0]: 0, grp[1]: 512}
                            for i, h in enumerate(grp):
                                xx[h] = xxt[i][:, 0:JW]
                        else:
                            # merged 4-head tail: concurrent pairs (0,1)
                            # then (2,3) land in different banks; bank
                            # pairing is (0,2) and (1,3)
                            colmap = {0: 0, 1: 512, 2: 256, 3: 768}
                            for h in grp:
                                xx[h] = xxt[h % 2][:, 256 * (h // 2):
                                                   256 * (h // 2) + JW]
                        pexp = [None] * NMT
                        tsum = [None] * NP
                        usum = [None] * 4

                        def emit_qk_exp(mt):
                            # heads 0,1 run concurrently in row groups 0,1
                            # (distinct banks); in the merged tail, heads
                            # 2,3 reuse row groups 0,1 (hi tiles) so they
                            # serialize behind 0,1 while filling the other
                            # half of each bank.
                            moff = 128 * mt
                            pe = pexpp.tile([128, 1024], BF16, name="pe",
                                            tag="pe", bufs=8)
                            st = stp.tile([128, 1024], F32, name="st",
                                          tag=f"st{mt % 2}")
                            for h in grp:
                                if G == 2 or h < 2:
                                    ks, qs, r0 = k_all, q_all, 32 * h
                                else:
                                    ks, qs, r0 = k_hi, q_hi, 32 * (h - 2)
                                nc.tensor.matmul(
                                    st[:, colmap[h]:colmap[h] + JW],
                                    ks[r0:r0 + 32, moff:moff + 128],
                                    qs[r0:r0 + 32, joff:joff + JW],
                                    start=True, stop=True,
                                    tile_position=(r0, 0))
                            if JW == 512 or G == 4:
                                nc.scalar.activation(pe[:, 0:1024],
                                                     st[:, 0:1024], AF.Exp)
                            else:
                                st_v = st.rearrange("p (a b) -> p a b",
                                                    b=512)[:, :, 0:JW]
                                pe_v = pe.rearrange("p (a b) -> p a b",
                                                    b=512)[:, :, 0:JW]
                                nc.scalar.activation(pe_v, st_v, AF.Exp)
                            pexp[mt] = pe

                        def emit_pv(mt, first=False, pexp=pexp, xx=xx,
                                    grp=grp, colmap=colmap, JW=JW, G=G):
                            # PV(1) executes first (psum reset); PV(0) joins
                            # late so the new pair's xx reset never waits on
                            # the previous pair's finish chain reading xx.
                            # start=True clears the whole bank's has_written
                            # bits, so only the first head per shared bank
                            # carries it (G==4).
                            pe = pexp[mt]
                            for h in grp:
                                nc.tensor.matmul(
                                    xx[h],
                                    vt_all[:, 512 * mt + 128 * h:
                                           512 * mt + 128 * (h + 1)],
                                    pe[:, colmap[h]:colmap[h] + JW],
                                    start=(first and (G == 2 or h < 2)),
                                    stop=(mt == NMT - 1),
                                    skip_group_check=True)

                        def emit_tree(k):
                            # u-level folds as soon as both t inputs are
                            # ready; each u-tile's ones-matmul column-sums
                            # follow immediately, spread through the loop
                            # to fill PE bubbles (the accumulating sums
                            # tiles are allocated at u0). t8 joins in the
                            # deferred finish.
                            if k % 2 == 1 and k < 8:
                                u = k // 2
                                t = workp.tile([128, 1024], BF16,
                                               name=f"u{u}", tag=f"u{u}",
                                               bufs=1)
                                nc.vector.tensor_tensor(
                                    t[:], tsum[k - 1][:], tsum[k][:], OP.add)
                                usum[u] = t

                        # PV schedule: PVs trail their exp by two steps so
                        # they never wait on the exp semaphore; the first
                        # PVs start later still, with a 2-per-step ramp.
                        FS = 4 if JW == 512 else 6
                        pv_order = [1, 2, 0] + list(range(3, NMT))
                        pv_sched = {}
                        done = 0
                        for _mt in range(FS, NMT):
                            target = min(NMT, _mt - 1)
                            n = min(2 if _mt > FS else 1,
                                    max(0, target - done))
                            if n:
                                pv_sched[_mt] = pv_order[done:done + n]
                                done += n
                        pv_left = pv_order[done:]

                        sums_hs = [None, None]
                        for mt in range(NMT):
                            if mt == 0 and fin_prev is not None:
                                # t8 add (VectorE only) first: by the time
                                # the PE-side finish runs at mt 1, t8 is
                                # done and the sums stop-matmuls don't
                                # stall ST(0)/ST(1) behind DVE latency
                                fin_prev[0]()
                            if mt == 1 and fin_prev is not None:
                                fin_prev[1]()
                                fin_prev = None
                            emit_qk_exp(mt)
                            if mt == 0 and carry is not None:
                                # previous pair's sums burst + trailing PVs
                                # run AFTER ST(0) so the exp chain (the
                                # pace-setter) is fed without a boundary
                                # pause
                                carry()
                                carry = None
                            if ci == 0 and gi == 0:
                                # v projection for m-tile mt, one step ahead
                                # of its PV consumer
                                ps_v = finp.tile([128, 512], F32, name="vps",
                                                 tag=f"f{mt % 2}")
                                for c2 in range(2):
                                    nc.tensor.matmul(
                                        ps_v[:],
                                        xf[c2][:, 128 * mt:128 * (mt + 1)],
                                        wvt[c2][:],
                                        start=(c2 == 0), stop=(c2 == 1))
                                nc.vector.tensor_copy(
                                    vt_all[:, 512 * mt:512 * (mt + 1)],
                                    ps_v[:])
                                qk_c = [(QSCHED, wqt, q_all, q_hi, bq_t),
                                        (KSCHED, wkt, k_all, k_hi, bk_t)]
                                for sched, wt, dst, dhi, bias in qk_c:
                                    if mt not in sched:
                                        continue
                                    qo = 512 * sched[mt]
                                    qw = min(512, N - qo)
                                    ps_q = finp.tile([128, 512], F32,
                                                     name="qps",
                                                     tag=f"f{(mt + 1) % 2}")
                                    for c2 in range(2):
                                        nc.tensor.matmul(
                                            ps_q[:, 0:qw], wt[c2][:],
                                            xf[c2][:, qo:qo + qw],
                                            start=(c2 == 0), stop=(c2 == 1))
                                    nc.vector.tensor_scalar_add(
                                        dst[:, qo:qo + qw],
                                        ps_q[:, 0:qw], bias[:])
                                    nc.vector.tensor_scalar_add(
                                        dhi[:, qo:qo + qw],
                                        ps_q[64:128, 0:qw],
                                        bias[64:128, :])
                            for j in pv_sched.get(mt, ()):
                                emit_pv(j, first=(j == 1))
                            if mt == NMT - 1 and is_last:
                                # final pair: burst in place
                                sw = JW if G == 2 else 512
                                for i in range(2):
                                    sums_hs[i] = finp.tile(
                                        [1, 512], F32, name="sums_h",
                                        tag=f"f{i % 2}")
                                    for uj in range(4):
                                        nc.tensor.matmul(
                                            sums_hs[i][:, 0:sw], ones_bf[:],
                                            usum[uj][:, 512 * i:512 * i + sw],
                                            start=(uj == 0), stop=False)
                            if mt == 3 and gi == 0 and pending is not None:
                                emit_proj(*pending)
                                pending = None
                            if mt % 2 == 1 and mt < NMT - 1:
                                k = mt // 2
                                t = workp.tile([128, 1024], BF16,
                                               name=f"t{k}", tag=f"t{k}",
                                               bufs=1)
                                nc.vector.tensor_tensor(
                                    t[:], pexp[mt - 1][:], pexp[mt][:],
                                    OP.add)
                                tsum[k] = t
                                emit_tree(k)
                        if is_last:
                            for j in pv_left:
                                emit_pv(j, first=(j == 1))
                        else:
                            def make_carry(emit_pv=emit_pv, usum=usum,
                                           sums_hs=sums_hs, JW=JW, G=G,
                                           pv_left=pv_left):
                                def cb():
                                    sw = JW if G == 2 else 512
                                    for i in range(2):
                                        sums_hs[i] = finp.tile(
                                            [1, 512], F32, name="sums_h",
                                            tag=f"f{i % 2}")
                                        for uj in range(4):
                                            nc.tensor.matmul(
                                                sums_hs[i][:, 0:sw],
                                                ones_bf[:],
                                                usum[uj][:, 512 * i:
                                                          512 * i + sw],
                                                start=(uj == 0), stop=False)
                                    for j in pv_left:
                                        emit_pv(j, first=(j == 1))
                                return cb
                            carry = make_carry()

                        def make_finish(grp=grp, xx=xx, xxt_c=xxt,
                                        pexp=pexp, sums_hs=sums_hs,
                                        r_out=r_ts, JW=JW, G=G):
                            cell = [None]

                            def fin_a():
                                # t8 add only (VectorE; no PE instruction)
                                t8 = workp.tile([128, 1024], BF16,
                                                name="t8", tag="t8", bufs=1)
                                nc.vector.tensor_tensor(
                                    t8[:], pexp[16][:], pexp[17][:], OP.add)
                                cell[0] = t8

                            def fin_b():
                                # denominators, normalize, bias+relu.
                                # Phases interleave the halves so the GPS
                                # broadcast of half 0 overlaps the DVE
                                # reciprocal of half 1.
                                sw = JW if G == 2 else 512
                                t8 = cell[0]
                                s_invs = [None, None]
                                bcs = [None, None]
                                for i in range(2):
                                    nc.tensor.matmul(
                                        sums_hs[i][:, 0:sw], ones_bf[:],
                                        t8[:, 512 * i:512 * i + sw],
                                        start=False, stop=True)
                                for i in range(2):
                                    s_inv = workp.tile([1, 512], F32,
                                                       name="s_inv",
                                                       tag=f"s_inv{i}")
                                    nc.vector.reciprocal_approx_fast(
                                        s_inv[:, 0:sw],
                                        sums_hs[i][:, 0:sw])
                                    s_invs[i] = s_inv
                                for i in range(2):
                                    bc = workp.tile([128, 512], F32,
                                                    name="inv_bc",
                                                    tag=f"inv_bc{i}")
                                    nc.gpsimd.partition_broadcast(
                                        bc[:, 0:sw], s_invs[i][:, 0:sw])
                                    bcs[i] = bc
                                for i in range(2):
                                    t_h = workp.tile([128, 512], F32,
                                                     name="t_h", tag="t_h")
                                    if G == 2:
                                        h = grp[i]
                                        nc.vector.tensor_tensor(
                                            t_h[:, 0:JW], xx[h],
                                            bcs[i][:, 0:JW], OP.mult)
                                        r_h = rp.tile([128, 512], FP16,
                                                      name=f"r{h}",
                                                      tag=f"r{h}")
                                        nc.vector.tensor_scalar(
                                            r_h[:, 0:JW], t_h[:, 0:JW],
                                            bv_t[h][:], 0.0, OP.add, OP.max)
                                        r_out[h] = r_h
                                    else:
                                        # bank i holds heads i (cols 0:256)
                                        # and i+2 (256:512)
                                        nc.vector.tensor_tensor(
                                            t_h[:, 0:512], xxt_c[i][:],
                                            bcs[i][:, 0:512], OP.mult)
                                        r_h = rp.tile([128, 512], FP16,
                                                      name=f"r{i}",
                                                      tag=f"r{i}")
                                        for hh in range(2):
                                            h = 2 * hh + i
                                            nc.vector.tensor_scalar(
                                                r_h[:, 256 * hh:
                                                    256 * hh + 256],
                                                t_h[:, 256 * hh:
                                                    256 * hh + 256],
                                                bv_t[h][:], 0.0,
                                                OP.add, OP.max)
                                            r_out[h] = r_h[:, 256 * hh:
                                                           256 * hh + 256]
                            return (fin_a, fin_b)

                        fin_prev = make_finish()
                        if last_in_chunk:
                            pending = (joff, JW, r_ts)
                fin_prev[0]()
                fin_prev[1]()
                emit_proj(*pending)
    nc.compile()
    return nc


def _prep_inputs(x, wq, gq, bq, wk, gk, bk, wv, gv, bv, wp, gp, bp):
    """Fold BN scales into weights; build the 8 per-core input maps."""
    rs = np.float32(1.0 / np.sqrt(np.float32(1.0) + np.float32(EPS)))
    sq = (gq * rs).astype(np.float32)
    sk = (gk * rs).astype(np.float32)
    sv = (gv * rs).astype(np.float32)
    sp = (gp * rs).astype(np.float32)
    wq_f = (wq * sq[:, None]).astype(np.float16)
    wk_f = (wk * sk[:, None]).astype(np.float16)
    wv_f = (wv * sv[:, None]).astype(np.float16)
    wp_f = (wp * sp[:, None]).astype(np.float16)

    xf = np.ascontiguousarray(x.reshape(B, C, N).astype(np.float16))
    consts = np.zeros((128, 128 + J), dtype=np.float32)
    consts[:, 0:128] = 1.0
    import ml_dtypes
    ones_bf = np.ones((128, 1), dtype=ml_dtypes.bfloat16)
    in_maps = []
    for core in range(8):
        b, g = core // 2, core % 2
        qs = slice(128 * g, 128 * (g + 1))       # q/k rows for this head group
        vs = slice(512 * g, 512 * (g + 1))       # v rows / p cols for this group
        in_maps.append({
            "x_in": xf[b],
            "wqt": np.ascontiguousarray(wq_f[qs, :].T),
            "wkt": np.ascontiguousarray(wk_f[qs, :].T),
            "wvt": np.ascontiguousarray(wv_f[vs, :].T),
            "wpt": np.ascontiguousarray(wp_f[:, vs].T),
            "bq": np.ascontiguousarray(bq[qs].astype(np.float32)[:, None]),
            "bk": np.ascontiguousarray(bk[qs].astype(np.float32)[:, None]),
            "bv": np.ascontiguousarray(bv[vs].astype(np.float32)[:, None]),
            "consts": consts,
            "ones_bf": ones_bf,
        })
    return in_maps


def kernel(**inputs):
    if "nc" not in _CACHE:
        _CACHE["nc"] = _build_program()
    nc = _CACHE["nc"]

    in_maps = _prep_inputs(**{k: np.asarray(v) for k, v in inputs.items()})
    res = run_bass_kernel_spmd(nc, in_maps, list(range(8)))
    _CACHE["last_results"] = res

    bp = np.asarray(inputs["bp"]).astype(np.float32)
    out = np.empty((B, C, H, W), dtype=np.float32)
    for b in range(B):
        acc = res.results[2 * b]["outp"] + res.results[2 * b + 1]["outp"]
        acc = acc + bp[:, None]
        out[b] = acc.reshape(C, H, W)
    return out
